# revision 1
# baseline (speedup 1.0000x reference)
import sys

sys.path.insert(0, "/opt/trn_rl_repo")
import numpy as np
from concourse import mybir
from concourse.bass import Bass, IndirectOffsetOnAxis
from concourse import bass_utils

C = 32
H = 1024  # final plane is (H, H, C) channel-last, flattened to (H*H+2, C)
NCORES = 8
GROUPS = 1968            # 128-point groups per core (multiple of 16)
NPC = GROUPS * 128       # points per core (incl. dummy tail)
BATCH = 16               # groups blended/stored together
NBATCH = GROUPS // BATCH

_f32 = mybir.dt.float32
_i32 = mybir.dt.int32


def _build_program():
    nc = Bass()
    g0s = nc.declare_dram_parameter("g0s", [128, 128, C], _f32, isOutput=False)
    g3s = nc.declare_dram_parameter("g3s", [3, 128, 128, C], _f32, isOutput=False)
    g2s = nc.declare_dram_parameter("g2s", [3, 256, 256, C], _f32, isOutput=False)
    g1s = nc.declare_dram_parameter("g1s", [3, 512, 512, C], _f32, isOutput=False)
    idx_d = nc.declare_dram_parameter("idx", [128, 2 * GROUPS], _i32, isOutput=False)
    w_d = nc.declare_dram_parameter("w", [128, 4 * GROUPS], _f32, isOutput=False)
    out_d = nc.declare_dram_parameter("out", [NPC, C], _f32, isOutput=True)

    ll2 = nc.dram_tensor("ll2", [256, 256, C], _f32)
    ll1 = nc.dram_tensor("ll1", [512, 512, C], _f32)
    plane = nc.dram_tensor("plane", [H * H + 2, C], _f32)

    dmaL = nc.alloc_semaphore()   # idwt input loads
    dmaP = nc.alloc_semaphore()   # idwt stores
    evW = nc.alloc_semaphore()    # idwt compute iters done
    dmaG = nc.alloc_semaphore()   # gather completions
    evB = nc.alloc_semaphore()    # blend batches done
    dmaO = nc.alloc_semaphore()   # output stores

    # ---- SBUF ----
    CB = 32 * C        # col-chunk free elems per coarse row (32 cols x 32 ch)
    inb = [[nc.alloc_sbuf_tensor(f"in{s}_{k}", [128, CB], _f32).ap()
            for k in range(4)] for s in range(2)]
    tmp = [[nc.alloc_sbuf_tensor(f"tmp{s}_{k}", [128, CB], _f32).ap()
            for k in range(4)] for s in range(2)]
    outb = [[nc.alloc_sbuf_tensor(f"out{s}_{k}", [128, 2 * CB], _f32).ap()
             for k in range(2)] for s in range(2)]
    idx_t = nc.alloc_sbuf_tensor("idx_t", [128, 2 * GROUPS], _i32).ap()
    w_t = nc.alloc_sbuf_tensor("w_t", [128, 4 * GROUPS], _f32).ap()
    Gb = [nc.alloc_sbuf_tensor(f"G{s}", [128, BATCH * 128], _f32).ap() for s in range(2)]
    Ob = [nc.alloc_sbuf_tensor(f"O{s}", [128, BATCH * C], _f32).ap() for s in range(2)]

    # ---- IDWT iteration table ----
    # level A: coarse 128 -> ll2 ; 1 row-tile x 4 col-chunks
    # level B: coarse 256 -> ll1 ; 2 row-tiles x 8 col-chunks
    # level C: coarse 512 -> plane ; 4 row-tiles x 16 col-chunks
    iters = []
    for cb in range(4):
        iters.append(("A", 0, cb))
    for r in range(2):
        for cb in range(8):
            iters.append(("B", r, cb))
    for r in range(4):
        for cb in range(16):
            iters.append(("C", r, cb))
    NIT = len(iters)

    planev = plane[: H * H].rearrange(
        "(rt p two w) c -> rt p two w c", rt=4, p=128, two=2, w=H
    )
    ll1v = ll1[:].rearrange("(rb p two) w c -> rb p two w c", rb=2, p=128)
    ll2v = ll2[:].rearrange("(p two) w c -> p two w c", two=2)

    def srcs(level, r, cb):
        cs = slice(cb * 32, cb * 32 + 32)
        if level == "A":
            return [g0s[:, cs, :]] + [g3s[b][:, cs, :] for b in range(3)]
        if level == "B":
            rs = slice(r * 128, r * 128 + 128)
            return [ll2[rs, cs, :]] + [g2s[b][rs, cs, :] for b in range(3)]
        rs = slice(r * 128, r * 128 + 128)
        return [ll1[rs, cs, :]] + [g1s[b][rs, cs, :] for b in range(3)]

    def dsts(level, r, cb):
        ws = slice(cb * 64, cb * 64 + 64)
        if level == "A":
            return [ll2v[:, 0, ws, :], ll2v[:, 1, ws, :]]
        if level == "B":
            return [ll1v[r, :, 0, ws, :], ll1v[r, :, 1, ws, :]]
        return [planev[r, :, 0, ws, :], planev[r, :, 1, ws, :]]

    nL = nP = nW = 0
    lvl_start = {"A": 0, "B": 4, "C": 4 + 16}
    for i, (level, r, cb) in enumerate(iters):
        s = i % 2
        # WAR: in/tmp/out bufs of this parity free after compute i-2 + stores i-2
        if i >= 2:
            nc.sync.wait_ge(evW, i - 1)
        # RAW level boundary: first load of B/C waits for prior level stores
        if i == lvl_start["B"]:
            nc.sync.wait_ge(dmaP, 32 * 4)
        if i == lvl_start["C"]:
            nc.sync.wait_ge(dmaP, 32 * (4 + 16))
        for k, src in enumerate(srcs(level, r, cb)):
            nc.sync.dma_start(
                out=inb[s][k][:].rearrange("p (w c) -> p w c", c=C), in_=src
            ).then_inc(dmaL, 16)
            nL += 16
        nc.vector.wait_ge(dmaL, nL)
        if i >= 2:
            nc.vector.wait_ge(dmaP, 32 * (i - 1))
        A = mybir.AluOpType.add
        S = mybir.AluOpType.subtract
        ll, lh, hl, hh = (b[:] for b in inb[s])
        t1, t2, t3, t4 = (b[:] for b in tmp[s])
        oE = outb[s][0][:].rearrange("p (w two c) -> p w two c", two=2, c=C)
        oO = outb[s][1][:].rearrange("p (w two c) -> p w two c", two=2, c=C)
        nc.vector.tensor_tensor(out=t1, in0=ll, in1=lh, op=S)
        nc.vector.tensor_tensor(out=t2, in0=ll, in1=lh, op=A)
        nc.vector.tensor_tensor(out=t3, in0=hl, in1=hh, op=S)
        nc.vector.tensor_tensor(out=t4, in0=hl, in1=hh, op=A)
        nc.vector.tensor_tensor(out=oE[:, :, 0, :], in0=t1, in1=t3, op=S)
        nc.vector.tensor_tensor(out=oE[:, :, 1, :], in0=t1, in1=t3, op=A)
        nc.vector.tensor_tensor(out=oO[:, :, 0, :], in0=t2, in1=t4, op=S)
        nc.vector.tensor_tensor(out=oO[:, :, 1, :], in0=t2, in1=t4, op=A).then_inc(
            evW, 1
        )
        nW += 1
        nc.sync.wait_ge(evW, nW)
        for k, dst in enumerate(dsts(level, r, cb)):
            nc.sync.dma_start(
                out=dst, in_=outb[s][k][:].rearrange("p (w c) -> p w c", c=C)
            ).then_inc(dmaP, 16)
            nP += 16

    # ---- gather + blend ----
    nc.sync.dma_start(out=idx_t[:], in_=idx_d[:]).then_inc(dmaL, 16)
    nc.sync.dma_start(out=w_t[:], in_=w_d[:]).then_inc(dmaL, 16)
    nL += 32

    nc.gpsimd.wait_ge(dmaP, nP)     # plane complete
    nc.gpsimd.wait_ge(dmaL, nL)     # idx loaded
    nc.vector.wait_ge(dmaL, nL)

    wv = w_t[:].rearrange("p (g j) -> p g j", j=4)
    outv = out_d[:].rearrange("(g p) c -> p g c", p=128)

    nG = 0
    nO = 0
    for b in range(NBATCH):
        s = b % 2
        if b >= 2:
            nc.gpsimd.wait_ge(evB, b - 1)   # Gb[s] free after blend b-2
        for gi in range(BATCH):
            g = b * BATCH + gi
            for half in range(2):
                nc.gpsimd.indirect_dma_start(
                    out=Gb[s][:, gi * 128 + half * 64 : gi * 128 + half * 64 + 64],
                    out_offset=None,
                    in_=plane[:],
                    in_offset=IndirectOffsetOnAxis(
                        ap=idx_t[:, 2 * g + half : 2 * g + half + 1], axis=0
                    ),
                ).then_inc(dmaG, 16)
                nG += 16
        nc.vector.wait_ge(dmaG, nG)
        if b >= 2:
            nc.vector.wait_ge(dmaO, 16 * (b - 1))  # Ob[s] stored
        Gv = Gb[s][:].rearrange("p (g s c) -> p g s c", s=4, c=C)
        Ov = Ob[s][:].rearrange("p (g c) -> p g c", c=C)
        gs = slice(b * BATCH, b * BATCH + BATCH)

        def wb(j):
            return wv[:, gs, j].unsqueeze(-1).broadcast_to([128, BATCH, C])

        M = mybir.AluOpType.mult
        A = mybir.AluOpType.add
        t1v = tmp[0][0][:].rearrange("p (g c) -> p g c", c=C)[:, :BATCH, :]
        t2v = tmp[0][1][:].rearrange("p (g c) -> p g c", c=C)[:, :BATCH, :]
        nc.vector.tensor_tensor(out=t1v, in0=Gv[:, :, 0, :], in1=wb(0), op=M)
        nc.vector.tensor_tensor(out=t2v, in0=Gv[:, :, 1, :], in1=wb(1), op=M)
        nc.vector.tensor_tensor(out=t1v, in0=t1v, in1=t2v, op=A)
        nc.vector.tensor_tensor(out=t2v, in0=Gv[:, :, 2, :], in1=wb(2), op=M)
        nc.vector.tensor_tensor(out=t1v, in0=t1v, in1=t2v, op=A)
        nc.vector.tensor_tensor(out=t2v, in0=Gv[:, :, 3, :], in1=wb(3), op=M)
        nc.vector.tensor_tensor(out=Ov, in0=t1v, in1=t2v, op=A).then_inc(evB, 1)
        nc.sync.wait_ge(evB, b + 1)
        nc.sync.dma_start(out=outv[:, gs, :], in_=Ob[s][:]).then_inc(dmaO, 16)
        nO += 16
    nc.sync.wait_ge(dmaO, nO)
    return nc


_NC_CACHE = None


def _get_nc():
    global _NC_CACHE
    if _NC_CACHE is None:
        _NC_CACHE = _build_program()
    return _NC_CACHE


def _prep_host(pts, g0, g1, g2, g3):
    f = np.float32
    # fold SCALER and the per-level 1/2 into the coefficients; channel-last
    g0s = np.ascontiguousarray(g0[0].transpose(1, 2, 0)) * f(0.125)
    g3s = np.ascontiguousarray(g3[0].transpose(1, 2, 3, 0)) * f(0.6 * 0.125)
    g2s = np.ascontiguousarray(g2[0].transpose(1, 2, 3, 0)) * f(0.4 * 0.25)
    g1s = np.ascontiguousarray(g1[0].transpose(1, 2, 3, 0)) * f(0.2 * 0.5)

    n = pts.shape[0]
    W1 = f(H - 1)
    x = np.clip((pts[:, 0] + f(1.0)) * f(0.5) * W1, f(0.0), W1)
    y = np.clip((pts[:, 1] + f(1.0)) * f(0.5) * W1, f(0.0), W1)
    x0 = np.floor(x)
    y0 = np.floor(y)
    wx = x - x0
    wy = y - y0
    x0i = x0.astype(np.int64)
    y0i = y0.astype(np.int64)
    x1c = np.minimum(x0i + 1, H - 1)
    y1c = np.minimum(y0i + 1, H - 1)
    # fetch starts at (y, x0); if x0 == H-1 shift left one pixel and swap weights
    shift = x0i == H - 1
    xs = np.where(shift, x0i - 1, x0i)
    wxa = np.where(shift, wx, f(1.0) - wx).astype(f)   # weight of pixel xs
    wxb = np.where(shift, f(1.0) - wx, wx).astype(f)   # weight of pixel xs+1
    # note: when shift, pixel xs+1 == x0 == x1 (clamped) carries full x weight 1
    wxa2 = np.where(shift, f(0.0), wxa)
    wxb2 = np.where(shift, f(1.0), wxb)
    idxA = (y0i * H + xs).astype(np.int32)
    idxB = (y1c * H + xs).astype(np.int32)
    wyA = (f(1.0) - wy).astype(f)
    w4 = np.stack(
        [wyA * wxa2, wyA * wxb2, wy.astype(f) * wxa2, wy.astype(f) * wxb2], axis=1
    )
    # spatial sort for gather locality
    order = np.argsort(idxA, kind="stable")
    idxA = idxA[order]
    idxB = idxB[order]
    w4 = w4[order]

    in_maps = []
    for c in range(NCORES):
        lo = c * NPC
        real = max(0, min(NPC, n - lo)) if lo < n else 0
        ia = np.zeros(NPC, np.int32)
        ib = np.zeros(NPC, np.int32)
        wc = np.zeros((NPC, 4), f)
        if real > 0:
            ia[:real] = idxA[lo : lo + real]
            ib[:real] = idxB[lo : lo + real]
            wc[:real] = w4[lo : lo + real]
        # device layout: point (g, p) = slab[g*128 + p]
        idx2 = np.empty((128, 2 * GROUPS), np.int32)
        idx2[:, 0::2] = ia.reshape(GROUPS, 128).T
        idx2[:, 1::2] = ib.reshape(GROUPS, 128).T
        wt = (
            wc.reshape(GROUPS, 128, 4)
            .transpose(1, 0, 2)
            .reshape(128, 4 * GROUPS)
            .astype(f)
        )
        in_maps.append(
            {
                "g0s": g0s.astype(f),
                "g3s": g3s.astype(f),
                "g2s": g2s.astype(f),
                "g1s": g1s.astype(f),
                "idx": idx2,
                "w": np.ascontiguousarray(wt),
            }
        )
    return in_maps, order, n


def kernel(pts, g0, g1, g2, g3, _res_hook=None):
    pts = np.asarray(pts, np.float32)
    nc = _get_nc()
    in_maps, order, n = _prep_host(
        pts, np.asarray(g0, np.float32), np.asarray(g1, np.float32),
        np.asarray(g2, np.float32), np.asarray(g3, np.float32),
    )
    res = bass_utils.run_bass_kernel_spmd(nc, in_maps, list(range(NCORES)))
    if _res_hook is not None:
        _res_hook(res)
    outs = np.concatenate([res.results[c]["out"] for c in range(NCORES)], axis=0)[:n]
    full = np.empty_like(outs)
    full[order] = outs
    return full



# revision 5
# speedup vs baseline: 2.1573x; 2.1573x over previous
import sys

sys.path.insert(0, "/opt/trn_rl_repo")
import numpy as np
from concourse import mybir
from concourse.bass import Bass, IndirectOffsetOnAxis
from concourse import bass_utils

C = 32
H = 1024
NCORES = 8
BATCH = 16            # groups per blend batch
CELLS = 64 * 512      # pair-cells per class table per core
SLAB = 144            # local plane rows computed per core (18 coarse rows)

_f16 = mybir.dt.float16
_i32 = mybir.dt.int32


def _build_program(gs):
    """gs: tuple of 4 group counts (classA/xr0, classA/xr1, classB/xr0, classB/xr1),
    each a multiple of BATCH. One SPMD program; per-core data via in_maps."""
    GTOT = sum(gs)
    nc = Bass()
    g0s = nc.declare_dram_parameter("g0s", [18, 128, C], _f16, isOutput=False)
    g3s = nc.declare_dram_parameter("g3s", [3, 18, 128, C], _f16, isOutput=False)
    g2s = nc.declare_dram_parameter("g2s", [3, 36, 256, C], _f16, isOutput=False)
    g1s = nc.declare_dram_parameter("g1s", [3, 72, 512, C], _f16, isOutput=False)
    idx_d = nc.declare_dram_parameter("idx", [128, GTOT], _i32, isOutput=False)
    w_d = nc.declare_dram_parameter("w", [128, 4 * GTOT], _f16, isOutput=False)
    out_d = nc.declare_dram_parameter("out", [GTOT * 128, C], _f16, isOutput=True)

    ll2 = nc.dram_tensor("ll2", [36, 256, C], _f16)
    ll1 = nc.dram_tensor("ll1", [72, 512, C], _f16)
    cA = nc.dram_tensor("cA", [CELLS, 2 * 2 * C], _f16)   # [k*512+u, (r,x,c)]
    cB = nc.dram_tensor("cB", [CELLS, 2 * 2 * C], _f16)

    dmaL = nc.alloc_semaphore()   # idwt input loads
    dmaP = nc.alloc_semaphore()   # idwt stores
    evW = nc.alloc_semaphore()    # idwt compute iters
    dmaW = nc.alloc_semaphore()   # idx/w loads
    dmaG = nc.alloc_semaphore()   # indirect gathers (must start at 0)
    evB = nc.alloc_semaphore()    # blend batches
    dmaO = nc.alloc_semaphore()   # output stores

    # ---- SBUF ----
    CHW = 64 * C                  # 64 output-col-pairs worth of one parity = 2048 els
    inb = [[nc.alloc_sbuf_tensor(f"in{s}_{k}", [128, CHW], _f16).ap()
            for k in range(4)] for s in range(2)]
    tmp = [[nc.alloc_sbuf_tensor(f"tmp{s}_{k}", [128, CHW], _f16).ap()
            for k in range(4)] for s in range(2)]
    # four output parities: (row-parity rp, col-parity xp)
    outb = [[nc.alloc_sbuf_tensor(f"out{s}_{k}", [128, CHW], _f16).ap()
             for k in range(4)] for s in range(2)]
    idx_t = nc.alloc_sbuf_tensor("idx_t", [128, GTOT], _i32).ap()
    w_t = nc.alloc_sbuf_tensor("w_t", [128, 4 * GTOT], _f16).ap()
    Gb = [nc.alloc_sbuf_tensor(f"G{s}", [128, BATCH * 256], _f16).ap() for s in range(2)]
    Ub = nc.alloc_sbuf_tensor("Ub", [128, BATCH * 4 * C], _f16).ap()
    U2 = nc.alloc_sbuf_tensor("U2", [128, BATCH * 2 * C], _f16).ap()
    Ob = [nc.alloc_sbuf_tensor(f"O{s}", [128, BATCH * C], _f16).ap() for s in range(2)]

    # ---- IDWT ----
    # levels: A: 18 coarse rows,128 wide -> ll2 (36,256); 2 col chunks of 64
    #         B: 36 rows,256 wide -> ll1 (72,512); 4 chunks
    #         C: 72 rows,512 wide -> cA/cB tables; 8 chunks
    iters = [("A", cb) for cb in range(2)] + [("B", cb) for cb in range(4)] + \
            [("C", cb) for cb in range(8)]
    NIT = len(iters)
    NPL = {"A": 18, "B": 36, "C": 72}
    ll2v = ll2[:].rearrange("(p two) (w xp) c -> p two w xp c", two=2, xp=2)
    ll1v = ll1[:].rearrange("(p two) (w xp) c -> p two w xp c", two=2, xp=2)
    cAv = cA[:].rearrange("(k u) (r x c) -> k u r x c", u=512, r=2, x=2)
    cBv = cB[:].rearrange("(k u) (r x c) -> k u r x c", u=512, r=2, x=2)

    def srcs(level, cb):
        cs = slice(cb * 64, cb * 64 + 64)
        if level == "A":
            return [g0s[:, cs, :]] + [g3s[b][:, cs, :] for b in range(3)]
        if level == "B":
            return [ll2[:, cs, :]] + [g2s[b][:, cs, :] for b in range(3)]
        return [ll1[:, cs, :]] + [g1s[b][:, cs, :] for b in range(3)]

    stores_per_iter = {"A": 4, "B": 4, "C": 8}
    cum_stores = []
    tot = 0
    for lv, _ in iters:
        cum_stores.append(tot)
        tot += stores_per_iter[lv] * 16
    TOT_STORES = tot
    lvlB_start, lvlC_start = 2, 6

    nL = nW = 0
    for i, (level, cb) in enumerate(iters):
        s = i % 2
        NP = NPL[level]
        if i >= 2:
            nc.sync.wait_ge(evW, i - 1)
        if i == lvlB_start:
            nc.sync.wait_ge(dmaP, cum_stores[lvlB_start])
        if i == lvlC_start:
            nc.sync.wait_ge(dmaP, cum_stores[lvlC_start])
        for k, src in enumerate(srcs(level, cb)):
            nc.sync.dma_start(
                out=inb[s][k][:NP].rearrange("p (w c) -> p w c", c=C), in_=src
            ).then_inc(dmaL, 16)
            nL += 16
        nc.vector.wait_ge(dmaL, nL)
        if i >= 2:
            nc.vector.wait_ge(dmaP, cum_stores[i - 1])  # stores of iter i-2 done
        A = mybir.AluOpType.add
        S = mybir.AluOpType.subtract
        ll, lh, hl, hh = (b[:NP] for b in inb[s])
        t1, t2, t3, t4 = (b[:NP] for b in tmp[s])
        oEE, oEO, oOE, oOO = (b[:NP] for b in outb[s])
        nc.vector.tensor_tensor(out=t1, in0=ll, in1=lh, op=S)   # row-even lo
        nc.vector.tensor_tensor(out=t2, in0=ll, in1=lh, op=A)   # row-odd lo
        nc.vector.tensor_tensor(out=t3, in0=hl, in1=hh, op=S)   # row-even hi
        nc.vector.tensor_tensor(out=t4, in0=hl, in1=hh, op=A)   # row-odd hi
        nc.vector.tensor_tensor(out=oEE, in0=t1, in1=t3, op=S)  # (2r, 2w)
        nc.vector.tensor_tensor(out=oEO, in0=t1, in1=t3, op=A)  # (2r, 2w+1)
        nc.vector.tensor_tensor(out=oOE, in0=t2, in1=t4, op=S)  # (2r+1, 2w)
        nc.vector.tensor_tensor(out=oOO, in0=t2, in1=t4, op=A).then_inc(evW, 1)
        nW += 1
        nc.sync.wait_ge(evW, nW)
        ws = slice(cb * 64, cb * 64 + 64)
        bufv = [b[:].rearrange("p (w c) -> p w c", c=C) for b in outb[s]]
        if level in ("A", "B"):
            dstv = ll2v if level == "A" else ll1v
            for bi, (rp, xp) in enumerate([(0, 0), (0, 1), (1, 0), (1, 1)]):
                nc.sync.dma_start(
                    out=dstv[:NP, rp, ws, xp, :], in_=bufv[bi][:NP]
                ).then_inc(dmaP, 16)
        else:
            # classA: pair k=p gets rows (2p, 2p+1); partitions 0..63
            for bi, (rp, xp) in enumerate([(0, 0), (0, 1), (1, 0), (1, 1)]):
                nc.sync.dma_start(
                    out=cAv[0:64, ws, rp, xp, :], in_=bufv[bi][0:64]
                ).then_inc(dmaP, 16)
            # classB r0 = odd rows (2p+1): k=p, partitions 0..63
            for bi, xp in [(2, 0), (3, 1)]:
                nc.sync.dma_start(
                    out=cBv[0:64, ws, 0, xp, :], in_=bufv[bi][0:64]
                ).then_inc(dmaP, 16)
            # classB r1 = even rows (2p), p=1..64 -> k=p-1
            for bi, xp in [(0, 0), (1, 1)]:
                nc.sync.dma_start(
                    out=cBv[0:64, ws, 1, xp, :], in_=bufv[bi][1:65]
                ).then_inc(dmaP, 16)

    # ---- gather + blend ----
    nc.sync.dma_start(out=idx_t[:], in_=idx_d[:]).then_inc(dmaW, 16)
    nc.sync.dma_start(out=w_t[:], in_=w_d[:]).then_inc(dmaW, 16)
    nc.gpsimd.wait_ge(dmaP, TOT_STORES)
    nc.gpsimd.wait_ge(dmaW, 32)
    nc.vector.wait_ge(dmaW, 32)

    wv = w_t[:].rearrange("p (g j) -> p g j", j=4)
    outv = out_d[:].rearrange("(g p) c -> p g c", p=128)
    M = mybir.AluOpType.mult
    A = mybir.AluOpType.add

    nG = 0
    nO = 0
    bglob = 0
    goff = 0
    for si, (tbl, xr) in enumerate([(cA, 0), (cA, 1), (cB, 0), (cB, 1)]):
        GS = gs[si]
        elen = 128 if xr == 0 else 256
        for b in range(GS // BATCH):
            s = bglob % 2
            if bglob >= 2:
                nc.gpsimd.wait_ge(evB, bglob - 1)
            for gi in range(BATCH):
                g = goff + b * BATCH + gi
                nc.gpsimd.indirect_dma_start(
                    out=Gb[s][:, gi * 256 : gi * 256 + elen],
                    out_offset=None,
                    in_=tbl[:],
                    in_offset=IndirectOffsetOnAxis(
                        ap=idx_t[:, g : g + 1], axis=0
                    ),
                ).then_inc(dmaG, 16)
                nG += 16
            nc.vector.wait_ge(dmaG, nG)
            if bglob >= 2:
                nc.vector.wait_ge(dmaO, 16 * (bglob - 1))
            gsl = slice(goff + b * BATCH, goff + b * BATCH + BATCH)
            # Gb viewed [p, g, cell(2), r(2), x(2), c]
            Gv = Gb[s][:].rearrange(
                "p (g cl r x c) -> p g cl r x c", cl=2, r=2, x=2, c=C
            )
            wv4 = w_t[:].rearrange("p (g r x) -> p g r x", r=2, x=2)
            U2v = U2[:].rearrange("p (g r c) -> p g r c", r=2, c=C)
            Ov = Ob[s][:].rearrange("p (g c) -> p g c", c=C)
            if xr == 0:
                Uv = Ub[:].rearrange("p (g r x c) -> p g r x c", r=2, x=2, c=C)
                Wb = (
                    wv4[:, gsl, :, :]
                    .unsqueeze(-1)
                    .broadcast_to([128, BATCH, 2, 2, C])
                )
                nc.vector.tensor_tensor(out=Uv, in0=Gv[:, :, 0], in1=Wb, op=M)
                nc.vector.tensor_tensor(
                    out=U2v, in0=Uv[:, :, :, 0, :], in1=Uv[:, :, :, 1, :], op=A
                )
            else:
                # x0 pixel = (cell0, x=1) with weight w[:, :, 0]; x0+1 = (cell1, x=0)
                GvA = Gv[:, :, 0, :, 1, :]
                GvB = Gv[:, :, 1, :, 0, :]
                WA = (
                    wv4[:, gsl, :, 0].unsqueeze(-1).broadcast_to([128, BATCH, 2, C])
                )
                WB = (
                    wv4[:, gsl, :, 1].unsqueeze(-1).broadcast_to([128, BATCH, 2, C])
                )
                Uv4 = Ub[:].rearrange("p (g r c) -> p g r c", r=4, c=C)
                T1 = Uv4[:, :, 0:2, :]
                T2 = Uv4[:, :, 2:4, :]
                nc.vector.tensor_tensor(out=T1, in0=GvA, in1=WA, op=M)
                nc.vector.tensor_tensor(out=T2, in0=GvB, in1=WB, op=M)
                nc.vector.tensor_tensor(out=U2v, in0=T1, in1=T2, op=A)
            nc.vector.tensor_tensor(
                out=Ov, in0=U2v[:, :, 0, :], in1=U2v[:, :, 1, :], op=A
            ).then_inc(evB, 1)
            nc.sync.wait_ge(evB, bglob + 1)
            nc.sync.dma_start(out=outv[:, gsl, :], in_=Ob[s][:]).then_inc(dmaO, 16)
            nO += 16
            bglob += 1
        goff += GS
    nc.sync.wait_ge(dmaO, nO)
    return nc


_NC_CACHE = {}


def _get_nc(gs):
    if gs not in _NC_CACHE:
        _NC_CACHE[gs] = _build_program(gs)
    return _NC_CACHE[gs]


def _prep_host(pts, g0, g1, g2, g3):
    f = np.float32
    g0s = np.ascontiguousarray(g0[0].transpose(1, 2, 0)) * f(0.125)
    g3s = np.ascontiguousarray(g3[0].transpose(1, 2, 3, 0)) * f(0.6 * 0.125)
    g2s = np.ascontiguousarray(g2[0].transpose(1, 2, 3, 0)) * f(0.4 * 0.25)
    g1s = np.ascontiguousarray(g1[0].transpose(1, 2, 3, 0)) * f(0.2 * 0.5)

    n = pts.shape[0]
    W1 = f(H - 1)
    x = np.clip((pts[:, 0] + f(1.0)) * f(0.5) * W1, f(0.0), W1)
    y = np.clip((pts[:, 1] + f(1.0)) * f(0.5) * W1, f(0.0), W1)
    x0 = np.floor(x)
    y0 = np.floor(y)
    wx = (x - x0).astype(f)
    wy = (y - y0).astype(f)
    x0i = x0.astype(np.int64)
    y0i = y0.astype(np.int64)
    sx = x0i == H - 1
    x0i = np.where(sx, x0i - 1, x0i)
    wx = np.where(sx, f(1.0), wx)
    sy = y0i == H - 1
    y0i = np.where(sy, y0i - 1, y0i)
    wy = np.where(sy, f(1.0), wy)

    core = (y0i >> 7).astype(np.int32)
    yl = (y0i & 127).astype(np.int32)
    P = yl & 1
    k = yl >> 1
    u = (x0i >> 1).astype(np.int32)
    xr = (x0i & 1).astype(np.int32)
    cell = k * 512 + u
    stream = P * 2 + xr
    w4 = np.stack(
        [(1 - wy) * (1 - wx), (1 - wy) * wx, wy * (1 - wx), wy * wx], axis=1
    ).astype(np.float16)

    order = np.lexsort((cell, stream, core))
    cell_s = cell[order]
    stream_s = stream[order]
    core_s = core[order]
    w4_s = w4[order]

    # per (core, stream) counts
    counts = np.zeros((NCORES, 4), np.int64)
    for c in range(NCORES):
        mc = core_s == c
        for s in range(4):
            counts[c, s] = int(np.sum(mc & (stream_s == s)))
    # SPMD: shared group counts per stream = max over cores, batch-rounded
    gs = tuple(
        int(-(-int(counts[:, s].max()) // (128 * BATCH)) * BATCH) for s in range(4)
    )
    GTOT = sum(gs)

    # coefficient slabs per core (zero-padded beyond grid)
    def slab(arr, r0, nr, full):
        if arr.ndim == 4:
            out = np.zeros((3, nr) + arr.shape[2:], np.float16)
            hi = min(full, r0 + nr)
            out[:, : hi - r0] = arr[:, r0:hi].astype(np.float16)
        else:
            out = np.zeros((nr,) + arr.shape[1:], np.float16)
            hi = min(full, r0 + nr)
            out[: hi - r0] = arr[r0:hi].astype(np.float16)
        return out

    in_maps = []
    dropped = []
    for c in range(NCORES):
        idx2 = np.zeros((128, GTOT), np.int32)
        wt = np.zeros((128, GTOT, 4), np.float16)
        goff = 0
        for s in range(4):
            sel = (core_s == c) & (stream_s == s)
            cells_cs = cell_s[sel]
            w_cs = w4_s[sel]
            cnt = cells_cs.shape[0]
            cap = gs[s] * 128
            assert cnt <= cap, f"stream overflow core {c} stream {s}"
            # point j -> group goff + j//128, partition j%128
            gidx = goff + np.arange(cnt) // 128
            pidx = np.arange(cnt) % 128
            idx2[pidx, gidx] = cells_cs
            wt[pidx, gidx] = w_cs
            goff += gs[s]
        in_maps.append(
            {
                "g0s": slab(g0s, 16 * c, 18, 128),
                "g3s": slab(g3s, 16 * c, 18, 128),
                "g2s": slab(g2s, 32 * c, 36, 256),
                "g1s": slab(g1s, 64 * c, 72, 512),
                "idx": idx2,
                "w": np.ascontiguousarray(wt.reshape(128, 4 * GTOT)),
            }
        )
    return in_maps, order, counts, gs, n


def kernel(pts, g0, g1, g2, g3, _res_hook=None):
    pts = np.asarray(pts, np.float32)
    in_maps, order, counts, gs, n = _prep_host(
        pts, np.asarray(g0, np.float32), np.asarray(g1, np.float32),
        np.asarray(g2, np.float32), np.asarray(g3, np.float32),
    )
    nc = _get_nc(gs)
    res = bass_utils.run_bass_kernel_spmd(nc, in_maps, list(range(NCORES)))
    if _res_hook is not None:
        _res_hook(res)
    out_sorted = np.empty((n, C), np.float32)
    pos = 0
    for c in range(NCORES):
        o = res.results[c]["out"]
        goff = 0
        for s in range(4):
            cnt = int(counts[c, s])
            out_sorted[pos : pos + cnt] = o[goff * 128 : goff * 128 + cnt].astype(
                np.float32
            )
            pos += cnt
            goff += gs[s]
    full = np.empty_like(out_sorted)
    full[order] = out_sorted
    return full


# revision 15
# speedup vs baseline: 5.0540x; 2.3428x over previous
import sys

sys.path.insert(0, "/opt/trn_rl_repo")
import numpy as np
from concourse import mybir
from concourse.bass import Bass, IndirectOffsetOnAxis
from concourse import bass_utils

C = 32
H = 1024
NCORES = 8
BATCH = 16            # groups per blend batch
CELLS = 64 * 512      # pair-cells per class table per core
SLAB = 144            # local plane rows computed per core (18 coarse rows)
K = 3                 # capacity slots per (cell, xr) in the sweep
J = 8                 # cell-blocks (of 128 cells) per sweep iteration
NSB = CELLS // (J * 128)   # sweep iterations per class table

_f16 = mybir.dt.float16
_i32 = mybir.dt.int32


def _build_program(gs):
    """gs: tuple of 4 group counts (classA/xr0, classA/xr1, classB/xr0, classB/xr1),
    each a multiple of BATCH. One SPMD program; per-core data via in_maps."""
    GTOT = sum(gs)
    nc = Bass()
    g0s = nc.declare_dram_parameter("g0s", [18, 128, C], _f16, isOutput=False)
    g3s = nc.declare_dram_parameter("g3s", [3, 18, 128, C], _f16, isOutput=False)
    g2s = nc.declare_dram_parameter("g2s", [3, 36, 256, C], _f16, isOutput=False)
    g1s = nc.declare_dram_parameter("g1s", [3, 72, 512, C], _f16, isOutput=False)
    idx_d = nc.declare_dram_parameter("idx", [128, GTOT], _i32, isOutput=False)
    w_d = nc.declare_dram_parameter("w", [128, 4 * GTOT], _f16, isOutput=False)
    # sweep slot weights: [p, cl, sb, j, xr, k, 4]
    WTOT = 2 * NSB * J * 2 * K * 4
    ws_d = nc.declare_dram_parameter("ws", [128, WTOT], _f16, isOutput=False)
    out_d = nc.declare_dram_parameter("out", [GTOT * 128, C], _f16, isOutput=True)
    outS_d = nc.declare_dram_parameter(
        "outS", [2 * CELLS, 2 * K * C], _f16, isOutput=True
    )

    ll2 = nc.dram_tensor("ll2", [36, 256, C], _f16)
    ll1 = nc.dram_tensor("ll1", [72, 512, C], _f16)
    cA = nc.dram_tensor("cA", [CELLS + 1, 2 * 2 * C], _f16)  # [k*512+u, (r,x,c)]
    cB = nc.dram_tensor("cB", [CELLS + 1, 2 * 2 * C], _f16)

    dmaL = nc.alloc_semaphore()   # idwt input loads
    dmaP = nc.alloc_semaphore()   # idwt stores
    evW = nc.alloc_semaphore()    # idwt compute iters
    dmaW = nc.alloc_semaphore()   # idx/w loads
    dmaG = nc.alloc_semaphore()   # indirect gathers (must start at 0)
    evB = nc.alloc_semaphore()    # blend batches
    dmaO = nc.alloc_semaphore()   # output stores
    dmaS = nc.alloc_semaphore()   # sweep table loads
    evS = nc.alloc_semaphore()    # sweep blend iterations
    dmaOS = nc.alloc_semaphore()  # sweep output stores

    # ---- SBUF ----
    CHW = 64 * C                  # 64 output-col-pairs worth of one parity = 2048 els
    inb = [[nc.alloc_sbuf_tensor(f"in{s}_{k}", [128, CHW], _f16).ap()
            for k in range(4)] for s in range(2)]
    tmp = [[nc.alloc_sbuf_tensor(f"tmp{s}_{k}", [128, CHW], _f16).ap()
            for k in range(4)] for s in range(2)]
    # four output parities: (row-parity rp, col-parity xp)
    outb = [[nc.alloc_sbuf_tensor(f"out{s}_{k}", [128, CHW], _f16).ap()
             for k in range(4)] for s in range(2)]
    idx_t = nc.alloc_sbuf_tensor("idx_t", [128, GTOT], _i32).ap()
    w_t = nc.alloc_sbuf_tensor("w_t", [128, 4 * GTOT], _f16).ap()
    Gb = [nc.alloc_sbuf_tensor(f"G{s}", [128, BATCH * 256], _f16).ap() for s in range(2)]
    Ub = nc.alloc_sbuf_tensor("Ub", [128, BATCH * 4 * C], _f16).ap()
    U2 = nc.alloc_sbuf_tensor("U2", [128, BATCH * 2 * C], _f16).ap()
    Ob = [nc.alloc_sbuf_tensor(f"O{s}", [128, BATCH * C], _f16).ap() for s in range(2)]
    ws_t = nc.alloc_sbuf_tensor("ws_t", [128, WTOT], _f16).ap()
    TL = [nc.alloc_sbuf_tensor(f"TL{s}", [128, J * 2 * 128], _f16).ap()
          for s in range(2)]
    UbS = nc.alloc_sbuf_tensor("UbS", [128, J * 4 * C], _f16).ap()
    U2S = nc.alloc_sbuf_tensor("U2S", [128, J * 2 * C], _f16).ap()
    ObS = [nc.alloc_sbuf_tensor(f"OS{s}", [128, J * 2 * K * C], _f16).ap()
           for s in range(2)]

    # ---- IDWT ----
    # levels: A: 18 coarse rows,128 wide -> ll2 (36,256); 2 col chunks of 64
    #         B: 36 rows,256 wide -> ll1 (72,512); 4 chunks
    #         C: 72 rows,512 wide -> cA/cB tables; 8 chunks
    iters = [("A", cb) for cb in range(2)] + [("B", cb) for cb in range(4)] + \
            [("C", cb) for cb in range(8)]
    NIT = len(iters)
    NPL = {"A": 18, "B": 36, "C": 72}
    ll2v = ll2[:].rearrange("(p two) (w xp) c -> p two w xp c", two=2, xp=2)
    ll1v = ll1[:].rearrange("(p two) (w xp) c -> p two w xp c", two=2, xp=2)
    cAv = cA[0:CELLS].rearrange("(k u) (r x c) -> k u r x c", u=512, r=2, x=2)
    cBv = cB[0:CELLS].rearrange("(k u) (r x c) -> k u r x c", u=512, r=2, x=2)

    def srcs(level, cb):
        cs = slice(cb * 64, cb * 64 + 64)
        if level == "A":
            return [g0s[:, cs, :]] + [g3s[b][:, cs, :] for b in range(3)]
        if level == "B":
            return [ll2[:, cs, :]] + [g2s[b][:, cs, :] for b in range(3)]
        return [ll1[:, cs, :]] + [g1s[b][:, cs, :] for b in range(3)]

    stores_per_iter = {"A": 4, "B": 4, "C": 8}
    cum_stores = []
    tot = 0
    for lv, _ in iters:
        cum_stores.append(tot)
        tot += stores_per_iter[lv] * 16
    TOT_STORES = tot
    lvlB_start, lvlC_start = 2, 6

    nL = nW = 0
    for i, (level, cb) in enumerate(iters):
        s = i % 2
        NP = NPL[level]
        if i >= 2:
            nc.sync.wait_ge(evW, i - 1)
        if i == lvlB_start:
            nc.sync.wait_ge(dmaP, cum_stores[lvlB_start])
        if i == lvlC_start:
            nc.sync.wait_ge(dmaP, cum_stores[lvlC_start])
        for k, src in enumerate(srcs(level, cb)):
            nc.sync.dma_start(
                out=inb[s][k][:NP].rearrange("p (w c) -> p w c", c=C), in_=src
            ).then_inc(dmaL, 16)
            nL += 16
        nc.vector.wait_ge(dmaL, nL)
        if i >= 2:
            nc.vector.wait_ge(dmaP, cum_stores[i - 1])  # stores of iter i-2 done
        A = mybir.AluOpType.add
        S = mybir.AluOpType.subtract
        ll, lh, hl, hh = (b[:NP] for b in inb[s])
        t1, t2, t3, t4 = (b[:NP] for b in tmp[s])
        oEE, oEO, oOE, oOO = (b[:NP] for b in outb[s])
        nc.vector.tensor_tensor(out=t1, in0=ll, in1=lh, op=S)   # row-even lo
        nc.vector.tensor_tensor(out=t2, in0=ll, in1=lh, op=A)   # row-odd lo
        nc.vector.tensor_tensor(out=t3, in0=hl, in1=hh, op=S)   # row-even hi
        nc.vector.tensor_tensor(out=t4, in0=hl, in1=hh, op=A)   # row-odd hi
        nc.vector.tensor_tensor(out=oEE, in0=t1, in1=t3, op=S)  # (2r, 2w)
        nc.vector.tensor_tensor(out=oEO, in0=t1, in1=t3, op=A)  # (2r, 2w+1)
        nc.vector.tensor_tensor(out=oOE, in0=t2, in1=t4, op=S)  # (2r+1, 2w)
        nc.vector.tensor_tensor(out=oOO, in0=t2, in1=t4, op=A).then_inc(evW, 1)
        nW += 1
        nc.sync.wait_ge(evW, nW)
        ws = slice(cb * 64, cb * 64 + 64)
        bufv = [b[:].rearrange("p (w c) -> p w c", c=C) for b in outb[s]]
        if level in ("A", "B"):
            dstv = ll2v if level == "A" else ll1v
            for bi, (rp, xp) in enumerate([(0, 0), (0, 1), (1, 0), (1, 1)]):
                nc.sync.dma_start(
                    out=dstv[:NP, rp, ws, xp, :], in_=bufv[bi][:NP]
                ).then_inc(dmaP, 16)
        else:
            # classA: pair k=p gets rows (2p, 2p+1); partitions 0..63
            for bi, (rp, xp) in enumerate([(0, 0), (0, 1), (1, 0), (1, 1)]):
                nc.sync.dma_start(
                    out=cAv[0:64, ws, rp, xp, :], in_=bufv[bi][0:64]
                ).then_inc(dmaP, 16)
            # classB r0 = odd rows (2p+1): k=p, partitions 0..63
            for bi, xp in [(2, 0), (3, 1)]:
                nc.sync.dma_start(
                    out=cBv[0:64, ws, 0, xp, :], in_=bufv[bi][0:64]
                ).then_inc(dmaP, 16)
            # classB r1 = even rows (2p), p=1..64 -> k=p-1
            for bi, xp in [(0, 0), (1, 1)]:
                nc.sync.dma_start(
                    out=cBv[0:64, ws, 1, xp, :], in_=bufv[bi][1:65]
                ).then_inc(dmaP, 16)

    # ---- sweep: K slots per (cell, xr), plain DMA loads, static-AP blends ----
    nc.sync.dma_start(out=idx_t[:], in_=idx_d[:]).then_inc(dmaW, 16)
    nc.sync.dma_start(out=w_t[:], in_=w_d[:]).then_inc(dmaW, 16)
    nc.sync.dma_start(out=ws_t[:], in_=ws_d[:]).then_inc(dmaW, 16)
    M = mybir.AluOpType.mult
    A = mybir.AluOpType.add
    wsv = ws_t[:].rearrange(
        "p (cl sb j xr k q) -> p cl sb j xr k q", cl=2, sb=NSB, j=J, xr=2, k=K
    )
    outSv = outS_d[:].rearrange(
        "(cl sb j p) w -> cl sb p j w", cl=2, sb=NSB, j=J, p=128
    )
    nc.scalar.wait_ge(dmaP, TOT_STORES)   # tables complete before sweep loads
    nc.vector.wait_ge(dmaW, 48)
    nOS = 0
    for t in range(2 * NSB):
        cl, sb = t // NSB, t % NSB
        tbl = cA if cl == 0 else cB
        own = tbl[0:CELLS].rearrange("(sb j p) e -> sb p j e", sb=NSB, j=J, p=128)
        nxt = tbl[1 : CELLS + 1].rearrange(
            "(sb j p) e -> sb p j e", sb=NSB, j=J, p=128
        )
        s2 = t % 2
        TLv = TL[s2][:].rearrange("p (j h e) -> p j h e", h=2, e=128)
        if t >= 2:
            nc.scalar.wait_ge(evS, t - 1)
        nc.scalar.dma_start(out=TLv[:, :, 0, :], in_=own[sb]).then_inc(dmaS, 16)
        nc.scalar.dma_start(out=TLv[:, :, 1, :], in_=nxt[sb]).then_inc(dmaS, 16)
        nc.vector.wait_ge(dmaS, 32 * (t + 1))
        if t >= 2:
            nc.vector.wait_ge(dmaOS, 16 * (t - 1))
        TLx = TL[s2][:].rearrange(
            "p (j h r x c) -> p j h r x c", h=2, r=2, x=2, c=C
        )
        OSv = ObS[s2][:].rearrange("p (j xr k c) -> p j xr k c", xr=2, k=K, c=C)
        UvS = UbS[:].rearrange("p (j r x c) -> p j r x c", r=2, x=2, c=C)
        Uv4S = UbS[:].rearrange("p (j r c) -> p j r c", r=4, c=C)
        U2vS = U2S[:].rearrange("p (j r c) -> p j r c", r=2, c=C)
        for xr in range(2):
            for k in range(K):
                Wk = wsv[:, cl, sb, :, xr, k, :]
                if xr == 0:
                    Wb = (
                        Wk.rearrange("p j (r x) -> p j r x", r=2, x=2)
                        .unsqueeze(-1)
                        .broadcast_to([128, J, 2, 2, C])
                    )
                    nc.vector.tensor_tensor(
                        out=UvS, in0=TLx[:, :, 0], in1=Wb, op=M
                    )
                    nc.vector.tensor_tensor(
                        out=U2vS, in0=UvS[:, :, :, 0, :], in1=UvS[:, :, :, 1, :],
                        op=A,
                    )
                else:
                    Wk4 = Wk.rearrange("p j (r x) -> p j r x", r=2, x=2)
                    WA = Wk4[:, :, :, 0].unsqueeze(-1).broadcast_to([128, J, 2, C])
                    WB = Wk4[:, :, :, 1].unsqueeze(-1).broadcast_to([128, J, 2, C])
                    nc.vector.tensor_tensor(
                        out=Uv4S[:, :, 0:2, :], in0=TLx[:, :, 0, :, 1, :], in1=WA,
                        op=M,
                    )
                    nc.vector.tensor_tensor(
                        out=Uv4S[:, :, 2:4, :], in0=TLx[:, :, 1, :, 0, :], in1=WB,
                        op=M,
                    )
                    nc.vector.tensor_tensor(
                        out=U2vS, in0=Uv4S[:, :, 0:2, :], in1=Uv4S[:, :, 2:4, :],
                        op=A,
                    )
                fin = nc.vector.tensor_tensor(
                    out=OSv[:, :, xr, k, :], in0=U2vS[:, :, 0, :],
                    in1=U2vS[:, :, 1, :], op=A,
                )
                if xr == 1 and k == K - 1:
                    fin.then_inc(evS, 1)
        nc.sync.wait_ge(evS, t + 1)
        nc.sync.dma_start(out=outSv[cl, sb], in_=ObS[s2][:]).then_inc(dmaOS, 16)
        nOS += 16
    nc.sync.wait_ge(dmaOS, nOS)

    # ---- leftover gather + blend ----
    nc.gpsimd.wait_ge(dmaP, TOT_STORES)
    nc.gpsimd.wait_ge(dmaW, 48)

    wv = w_t[:].rearrange("p (g j) -> p g j", j=4)
    outv = out_d[:].rearrange("(g p) c -> p g c", p=128)
    M = mybir.AluOpType.mult
    A = mybir.AluOpType.add

    nG = 0
    nO = 0
    bglob = 0
    goff = 0
    for si, (tbl, xr) in enumerate([(cA, 0), (cA, 1), (cB, 0), (cB, 1)]):
        GS = gs[si]
        elen = 128 if xr == 0 else 256
        for b in range(GS // BATCH):
            s = bglob % 2
            if bglob >= 2:
                nc.gpsimd.wait_ge(evB, bglob - 1)
            for gi in range(BATCH):
                g = goff + b * BATCH + gi
                nc.gpsimd.indirect_dma_start(
                    out=Gb[s][:, gi * 256 : gi * 256 + elen],
                    out_offset=None,
                    in_=tbl[:],
                    in_offset=IndirectOffsetOnAxis(
                        ap=idx_t[:, g : g + 1], axis=0
                    ),
                ).then_inc(dmaG, 16)
                nG += 16
            nc.vector.wait_ge(dmaG, nG)
            if bglob >= 2:
                nc.vector.wait_ge(dmaO, 16 * (bglob - 1))
            gsl = slice(goff + b * BATCH, goff + b * BATCH + BATCH)
            # Gb viewed [p, g, cell(2), r(2), x(2), c]
            Gv = Gb[s][:].rearrange(
                "p (g cl r x c) -> p g cl r x c", cl=2, r=2, x=2, c=C
            )
            wv4 = w_t[:].rearrange("p (g r x) -> p g r x", r=2, x=2)
            U2v = U2[:].rearrange("p (g r c) -> p g r c", r=2, c=C)
            Ov = Ob[s][:].rearrange("p (g c) -> p g c", c=C)
            if xr == 0:
                Uv = Ub[:].rearrange("p (g r x c) -> p g r x c", r=2, x=2, c=C)
                Wb = (
                    wv4[:, gsl, :, :]
                    .unsqueeze(-1)
                    .broadcast_to([128, BATCH, 2, 2, C])
                )
                nc.vector.tensor_tensor(out=Uv, in0=Gv[:, :, 0], in1=Wb, op=M)
                nc.vector.tensor_tensor(
                    out=U2v, in0=Uv[:, :, :, 0, :], in1=Uv[:, :, :, 1, :], op=A
                )
            else:
                # x0 pixel = (cell0, x=1) with weight w[:, :, 0]; x0+1 = (cell1, x=0)
                GvA = Gv[:, :, 0, :, 1, :]
                GvB = Gv[:, :, 1, :, 0, :]
                WA = (
                    wv4[:, gsl, :, 0].unsqueeze(-1).broadcast_to([128, BATCH, 2, C])
                )
                WB = (
                    wv4[:, gsl, :, 1].unsqueeze(-1).broadcast_to([128, BATCH, 2, C])
                )
                Uv4 = Ub[:].rearrange("p (g r c) -> p g r c", r=4, c=C)
                T1 = Uv4[:, :, 0:2, :]
                T2 = Uv4[:, :, 2:4, :]
                nc.vector.tensor_tensor(out=T1, in0=GvA, in1=WA, op=M)
                nc.vector.tensor_tensor(out=T2, in0=GvB, in1=WB, op=M)
                nc.vector.tensor_tensor(out=U2v, in0=T1, in1=T2, op=A)
            nc.vector.tensor_tensor(
                out=Ov, in0=U2v[:, :, 0, :], in1=U2v[:, :, 1, :], op=A
            ).then_inc(evB, 1)
            nc.sync.wait_ge(evB, bglob + 1)
            nc.sync.dma_start(out=outv[:, gsl, :], in_=Ob[s][:]).then_inc(dmaO, 16)
            nO += 16
            bglob += 1
        goff += GS
    nc.sync.wait_ge(dmaO, nO)
    return nc


_NC_CACHE = {}


def _get_nc(gs):
    if gs not in _NC_CACHE:
        _NC_CACHE[gs] = _build_program(gs)
    return _NC_CACHE[gs]


def _prep_host(pts, g0, g1, g2, g3):
    f = np.float32
    g0s = np.ascontiguousarray(g0[0].transpose(1, 2, 0)) * f(0.125)
    g3s = np.ascontiguousarray(g3[0].transpose(1, 2, 3, 0)) * f(0.6 * 0.125)
    g2s = np.ascontiguousarray(g2[0].transpose(1, 2, 3, 0)) * f(0.4 * 0.25)
    g1s = np.ascontiguousarray(g1[0].transpose(1, 2, 3, 0)) * f(0.2 * 0.5)

    n = pts.shape[0]
    W1 = f(H - 1)
    x = np.clip((pts[:, 0] + f(1.0)) * f(0.5) * W1, f(0.0), W1)
    y = np.clip((pts[:, 1] + f(1.0)) * f(0.5) * W1, f(0.0), W1)
    x0 = np.floor(x)
    y0 = np.floor(y)
    wx = (x - x0).astype(f)
    wy = (y - y0).astype(f)
    x0i = x0.astype(np.int64)
    y0i = y0.astype(np.int64)
    sx = x0i == H - 1
    x0i = np.where(sx, x0i - 1, x0i)
    wx = np.where(sx, f(1.0), wx)
    sy = y0i == H - 1
    y0i = np.where(sy, y0i - 1, y0i)
    wy = np.where(sy, f(1.0), wy)

    core = (y0i >> 7).astype(np.int32)
    yl = (y0i & 127).astype(np.int32)
    P = yl & 1
    k = yl >> 1
    u = (x0i >> 1).astype(np.int32)
    xr = (x0i & 1).astype(np.int32)
    cell = k * 512 + u
    stream = P * 2 + xr
    w4 = np.stack(
        [(1 - wy) * (1 - wx), (1 - wy) * wx, wy * (1 - wx), wy * wx], axis=1
    ).astype(np.float16)

    order = np.lexsort((cell, stream, core))
    cell_s = cell[order]
    stream_s = stream[order]
    core_s = core[order]
    w4_s = w4[order]
    P_s = stream_s >> 1
    xr_s = stream_s & 1

    # rank within (core, P, cell, xr) bin; first K go to sweep slots
    nn = cell_s.shape[0]
    binid = (((core_s.astype(np.int64) * 2 + P_s) * CELLS + cell_s) * 2 + xr_s)
    newb = np.empty(nn, bool)
    newb[0] = True
    newb[1:] = binid[1:] != binid[:-1]
    first = np.maximum.accumulate(np.where(newb, np.arange(nn), 0))
    rank = (np.arange(nn) - first).astype(np.int32)
    slot = rank < K

    # per (core, stream) leftover counts
    counts = np.zeros((NCORES, 4), np.int64)
    for c in range(NCORES):
        mc = (core_s == c) & ~slot
        for s in range(4):
            counts[c, s] = int(np.sum(mc & (stream_s == s)))
    # SPMD: shared group counts per stream = max over cores, batch-rounded
    gs = tuple(
        max(BATCH,
            int(-(-int(counts[:, s].max()) // (128 * BATCH)) * BATCH))
        for s in range(4)
    )
    GTOT = sum(gs)

    # coefficient slabs per core (zero-padded beyond grid)
    def slab(arr, r0, nr, full):
        if arr.ndim == 4:
            out = np.zeros((3, nr) + arr.shape[2:], np.float16)
            hi = min(full, r0 + nr)
            out[:, : hi - r0] = arr[:, r0:hi].astype(np.float16)
        else:
            out = np.zeros((nr,) + arr.shape[1:], np.float16)
            hi = min(full, r0 + nr)
            out[: hi - r0] = arr[r0:hi].astype(np.float16)
        return out

    in_maps = []
    for c in range(NCORES):
        idx2 = np.zeros((128, GTOT), np.int32)
        wt = np.zeros((128, GTOT, 4), np.float16)
        # sweep slot weights [p, cl, sb, j, xr, k, 4]
        wS = np.zeros((128, 2, NSB, J, 2, K, 4), np.float16)
        msl = (core_s == c) & slot
        ce = cell_s[msl]
        wS[ce & 127, P_s[msl], ce >> 10, (ce >> 7) & 7, xr_s[msl], rank[msl]] = (
            w4_s[msl]
        )
        goff = 0
        for s in range(4):
            sel = (core_s == c) & (stream_s == s) & ~slot
            cells_cs = cell_s[sel]
            w_cs = w4_s[sel]
            cnt = cells_cs.shape[0]
            cap = gs[s] * 128
            assert cnt <= cap, f"stream overflow core {c} stream {s}"
            # point j -> group goff + j//128, partition j%128
            gidx = goff + np.arange(cnt) // 128
            pidx = np.arange(cnt) % 128
            idx2[pidx, gidx] = cells_cs
            wt[pidx, gidx] = w_cs
            goff += gs[s]
        in_maps.append(
            {
                "g0s": slab(g0s, 16 * c, 18, 128),
                "g3s": slab(g3s, 16 * c, 18, 128),
                "g2s": slab(g2s, 32 * c, 36, 256),
                "g1s": slab(g1s, 64 * c, 72, 512),
                "idx": idx2,
                "w": np.ascontiguousarray(wt.reshape(128, 4 * GTOT)),
                "ws": np.ascontiguousarray(wS.reshape(128, -1)),
            }
        )
    return in_maps, order, counts, gs, n, (core_s, P_s, xr_s, cell_s, rank, slot)


def kernel(pts, g0, g1, g2, g3, _res_hook=None):
    pts = np.asarray(pts, np.float32)
    in_maps, order, counts, gs, n, meta = _prep_host(
        pts, np.asarray(g0, np.float32), np.asarray(g1, np.float32),
        np.asarray(g2, np.float32), np.asarray(g3, np.float32),
    )
    core_s, P_s, xr_s, cell_s, rank, slot = meta
    nc = _get_nc(gs)
    res = bass_utils.run_bass_kernel_spmd(nc, in_maps, list(range(NCORES)))
    if _res_hook is not None:
        _res_hook(res)
    out_sorted = np.empty((n, C), np.float32)
    for c in range(NCORES):
        mc = core_s == c
        # sweep-slotted points
        oS = res.results[c]["outS"].reshape(2, CELLS, 2, K, C)
        msl = mc & slot
        out_sorted[msl] = oS[
            P_s[msl], cell_s[msl], xr_s[msl], rank[msl]
        ].astype(np.float32)
        # leftover points, packed per stream in sorted order
        o = res.results[c]["out"]
        goff = 0
        for s in range(4):
            sel = mc & ~slot & ((P_s * 2 + xr_s) == s)
            cnt = int(counts[c, s])
            out_sorted[sel] = o[goff * 128 : goff * 128 + cnt].astype(np.float32)
            goff += gs[s]
    full = np.empty_like(out_sorted)
    full[order] = out_sorted
    return full


# revision 18
# speedup vs baseline: 5.3102x; 1.0507x over previous
import sys

sys.path.insert(0, "/opt/trn_rl_repo")
import numpy as np
from concourse import mybir
from concourse.bass import Bass, IndirectOffsetOnAxis
from concourse import bass_utils

C = 32
H = 1024
NCORES = 8
BATCH = 16            # groups per blend batch
CELLS = 64 * 512      # pair-cells per class table per core
SLAB = 144            # local plane rows computed per core (18 coarse rows)
K = 3                 # capacity slots per (cell, xr) in the sweep
J = 8                 # cell-blocks (of 128 cells) per sweep iteration
NSB = CELLS // (J * 128)   # sweep iterations per class table

_f16 = mybir.dt.float16
_i32 = mybir.dt.int32


def _build_program(gs):
    """gs: tuple of 4 group counts (classA/xr0, classA/xr1, classB/xr0, classB/xr1),
    each a multiple of BATCH. One SPMD program; per-core data via in_maps."""
    GTOT = sum(gs)
    nc = Bass()
    gA = nc.declare_dram_parameter("gA", [4, 18, 128, C], _f16, isOutput=False)
    g2s = nc.declare_dram_parameter("g2s", [3, 36, 256, C], _f16, isOutput=False)
    g1s = nc.declare_dram_parameter("g1s", [3, 72, 512, C], _f16, isOutput=False)
    idx_d = nc.declare_dram_parameter("idx", [128, GTOT], _i32, isOutput=False)
    w_d = nc.declare_dram_parameter("w", [128, 4 * GTOT], _f16, isOutput=False)
    # sweep slot weights: [p, cl, sb, j, xr, k, 4]
    WTOT = 2 * NSB * J * 2 * K * 4
    ws_d = nc.declare_dram_parameter("ws", [128, WTOT], _f16, isOutput=False)
    out_d = nc.declare_dram_parameter("out", [GTOT * 128, C], _f16, isOutput=True)
    outS_d = nc.declare_dram_parameter(
        "outS", [2 * CELLS, 2 * K * C], _f16, isOutput=True
    )

    ll2b = nc.dram_tensor("ll2b", [4, 36, 256, C], _f16)
    ll1b = nc.dram_tensor("ll1b", [4, 72, 512, C], _f16)
    cA = nc.dram_tensor("cA", [CELLS + 1, 2 * 2 * C], _f16)  # [k*512+u, (r,x,c)]
    cB = nc.dram_tensor("cB", [CELLS + 1, 2 * 2 * C], _f16)

    dmaL = nc.alloc_semaphore()   # idwt input loads
    dmaP = nc.alloc_semaphore()   # idwt stores
    evW = nc.alloc_semaphore()    # idwt compute iters
    dmaW = nc.alloc_semaphore()   # idx/w loads
    dmaG = nc.alloc_semaphore()   # indirect gathers (must start at 0)
    evB = nc.alloc_semaphore()    # blend batches
    dmaO = nc.alloc_semaphore()   # output stores
    dmaS = nc.alloc_semaphore()   # sweep table loads
    evS = nc.alloc_semaphore()    # sweep blend iterations
    dmaOS = nc.alloc_semaphore()  # sweep output stores

    # ---- SBUF ----
    CHW = 64 * C                  # 64 output-col-pairs worth of one parity = 2048 els
    inb = [nc.alloc_sbuf_tensor(f"in{s}", [128, 4 * CHW], _f16).ap()
           for s in range(2)]
    tmp = [[nc.alloc_sbuf_tensor(f"tmp{s}_{k}", [128, CHW], _f16).ap()
            for k in range(4)] for s in range(2)]
    # out bigbuf layout [p, w(64), rp(2), xp(2), c]
    outb = [nc.alloc_sbuf_tensor(f"out{s}", [128, 4 * CHW], _f16).ap()
            for s in range(2)]
    idx_t = nc.alloc_sbuf_tensor("idx_t", [128, GTOT], _i32).ap()
    w_t = nc.alloc_sbuf_tensor("w_t", [128, 4 * GTOT], _f16).ap()
    Gb = [nc.alloc_sbuf_tensor(f"G{s}", [128, BATCH * 256], _f16).ap() for s in range(2)]
    Ub = nc.alloc_sbuf_tensor("Ub", [128, BATCH * 4 * C], _f16).ap()
    U2 = nc.alloc_sbuf_tensor("U2", [128, BATCH * 2 * C], _f16).ap()
    Ob = [nc.alloc_sbuf_tensor(f"O{s}", [128, BATCH * C], _f16).ap() for s in range(2)]
    ws_t = nc.alloc_sbuf_tensor("ws_t", [128, WTOT], _f16).ap()
    TL = [nc.alloc_sbuf_tensor(f"TL{s}", [128, J * 2 * 128], _f16).ap()
          for s in range(2)]
    UbS = nc.alloc_sbuf_tensor("UbS", [128, J * 4 * C], _f16).ap()
    U2S = nc.alloc_sbuf_tensor("U2S", [128, J * 2 * C], _f16).ap()
    ObS = [nc.alloc_sbuf_tensor(f"OS{s}", [128, J * 2 * K * C], _f16).ap()
           for s in range(2)]

    # ---- IDWT ----
    # levels: A: 18 coarse rows,128 wide -> ll2 (36,256); 2 col chunks of 64
    #         B: 36 rows,256 wide -> ll1 (72,512); 4 chunks
    #         C: 72 rows,512 wide -> cA/cB tables; 8 chunks
    iters = [("A", cb) for cb in range(2)] + [("B", cb) for cb in range(4)] + \
            [("C", cb) for cb in range(8)]
    NIT = len(iters)
    NPL = {"A": 18, "B": 36, "C": 72}
    ll2v = ll2b[0].rearrange("(p two) (w xp) c -> p two w xp c", two=2, xp=2)
    ll1v = ll1b[0].rearrange("(p two) (w xp) c -> p two w xp c", two=2, xp=2)
    cAv = cA[0:CELLS].rearrange("(k u) (r x c) -> k u r x c", u=512, r=2, x=2)
    cBv = cB[0:CELLS].rearrange("(k u) (r x c) -> k u r x c", u=512, r=2, x=2)

    def src_packed(level, cb):
        cs = slice(cb * 64, cb * 64 + 64)
        if level == "A":
            return gA[:, :, cs, :].rearrange("b p w c -> p b w c")
        if level == "B":
            return ll2b[:, :, cs, :].rearrange("b p w c -> p b w c")
        return ll1b[:, :, cs, :].rearrange("b p w c -> p b w c")

    stores_per_iter = {"A": 4, "B": 4, "C": 3}
    cum_stores = []
    tot = 0
    for lv, _ in iters:
        cum_stores.append(tot)
        tot += stores_per_iter[lv] * 16
    TOT_STORES = tot
    lvlB_start, lvlC_start = 2, 6

    nc.sync.dma_start(out=ll2b[1:4], in_=g2s[:]).then_inc(dmaL, 16)
    nc.sync.dma_start(out=ll1b[1:4], in_=g1s[:]).then_inc(dmaL, 16)
    nL = 32
    nW = 0
    for i, (level, cb) in enumerate(iters):
        s = i % 2
        NP = NPL[level]
        if i >= 2:
            nc.sync.wait_ge(evW, i - 1)
        if i == lvlB_start:
            nc.sync.wait_ge(dmaP, cum_stores[lvlB_start])
            nc.sync.wait_ge(dmaL, 32 + 16 * lvlB_start)  # band copies done
        if i == lvlC_start:
            nc.sync.wait_ge(dmaP, cum_stores[lvlC_start])
        nc.sync.dma_start(
            out=inb[s][:NP].rearrange("p (b w c) -> p b w c", b=4, c=C),
            in_=src_packed(level, cb),
        ).then_inc(dmaL, 16)
        nL += 16
        nc.vector.wait_ge(dmaL, nL)
        if i >= 2:
            nc.vector.wait_ge(dmaP, cum_stores[i - 1])  # stores of iter i-2 done
        A = mybir.AluOpType.add
        S = mybir.AluOpType.subtract
        inv = inb[s][:NP].rearrange("p (b e) -> p b e", b=4)
        ll, lh, hl, hh = (inv[:, k] for k in range(4))
        t1, t2, t3, t4 = (b[:NP] for b in tmp[s])
        ov = outb[s][:NP].rearrange("p (w rp xp c) -> p w rp xp c", rp=2, xp=2, c=C)
        oEE = ov[:, :, 0, 0, :]
        oEO = ov[:, :, 0, 1, :]
        oOE = ov[:, :, 1, 0, :]
        oOO = ov[:, :, 1, 1, :]
        nc.vector.tensor_tensor(out=t1, in0=ll, in1=lh, op=S)   # row-even lo
        nc.vector.tensor_tensor(out=t2, in0=ll, in1=lh, op=A)   # row-odd lo
        nc.vector.tensor_tensor(out=t3, in0=hl, in1=hh, op=S)   # row-even hi
        nc.vector.tensor_tensor(out=t4, in0=hl, in1=hh, op=A)   # row-odd hi
        nc.vector.tensor_tensor(out=oEE, in0=t1, in1=t3, op=S)  # (2r, 2w)
        nc.vector.tensor_tensor(out=oEO, in0=t1, in1=t3, op=A)  # (2r, 2w+1)
        nc.vector.tensor_tensor(out=oOE, in0=t2, in1=t4, op=S)  # (2r+1, 2w)
        nc.vector.tensor_tensor(out=oOO, in0=t2, in1=t4, op=A).then_inc(evW, 1)
        nW += 1
        nc.sync.wait_ge(evW, nW)
        ws = slice(cb * 64, cb * 64 + 64)
        bufv = outb[s][:].rearrange("p (w rp xp c) -> p w rp xp c", rp=2, xp=2, c=C)
        if level in ("A", "B"):
            dstv = ll2v if level == "A" else ll1v
            for rp in range(2):
                for xp in range(2):
                    nc.sync.dma_start(
                        out=dstv[:NP, rp, ws, xp, :],
                        in_=bufv[:NP, :, rp, xp, :],
                    ).then_inc(dmaP, 16)
        else:
            # classA: pair k=p rows (2p, 2p+1): full cells, contiguous
            nc.sync.dma_start(
                out=cAv[0:64, ws, :, :, :], in_=bufv[0:64]
            ).then_inc(dmaP, 16)
            # classB r0 = odd rows (2p+1): k=p
            nc.sync.dma_start(
                out=cBv[0:64, ws, 0, :, :], in_=bufv[0:64, :, 1, :, :]
            ).then_inc(dmaP, 16)
            # classB r1 = even rows (2p), p=1..64 -> k=p-1
            nc.sync.dma_start(
                out=cBv[0:64, ws, 1, :, :], in_=bufv[1:65, :, 0, :, :]
            ).then_inc(dmaP, 16)

    # ---- sweep: K slots per (cell, xr), plain DMA loads, static-AP blends ----
    nc.sync.dma_start(out=idx_t[:], in_=idx_d[:]).then_inc(dmaW, 16)
    nc.sync.dma_start(out=w_t[:], in_=w_d[:]).then_inc(dmaW, 16)
    nc.sync.dma_start(out=ws_t[:], in_=ws_d[:]).then_inc(dmaW, 16)
    M = mybir.AluOpType.mult
    A = mybir.AluOpType.add
    wsv = ws_t[:].rearrange(
        "p (cl sb j xr k q) -> p cl sb j xr k q", cl=2, sb=NSB, j=J, xr=2, k=K
    )
    outSv = outS_d[:].rearrange(
        "(cl sb j p) w -> cl sb p j w", cl=2, sb=NSB, j=J, p=128
    )
    nc.scalar.wait_ge(dmaP, TOT_STORES)   # tables complete before sweep loads
    nc.vector.wait_ge(dmaW, 48)
    nOS = 0
    for t in range(2 * NSB):
        cl, sb = t // NSB, t % NSB
        tbl = cA if cl == 0 else cB
        own = tbl[0:CELLS].rearrange("(sb j p) e -> sb p j e", sb=NSB, j=J, p=128)
        nxt = tbl[1 : CELLS + 1].rearrange(
            "(sb j p) e -> sb p j e", sb=NSB, j=J, p=128
        )
        s2 = t % 2
        TLv = TL[s2][:].rearrange("p (j h e) -> p j h e", h=2, e=128)
        if t >= 2:
            nc.scalar.wait_ge(evS, t - 1)
        nc.scalar.dma_start(out=TLv[:, :, 0, :], in_=own[sb]).then_inc(dmaS, 16)
        nc.scalar.dma_start(out=TLv[:, :, 1, :], in_=nxt[sb]).then_inc(dmaS, 16)
        nc.vector.wait_ge(dmaS, 32 * (t + 1))
        if t >= 2:
            nc.vector.wait_ge(dmaOS, 16 * (t - 1))
        TLx = TL[s2][:].rearrange(
            "p (j h r x c) -> p j h r x c", h=2, r=2, x=2, c=C
        )
        OSv = ObS[s2][:].rearrange("p (j xr k c) -> p j xr k c", xr=2, k=K, c=C)
        UvS = UbS[:].rearrange("p (j r x c) -> p j r x c", r=2, x=2, c=C)
        Uv4S = UbS[:].rearrange("p (j r c) -> p j r c", r=4, c=C)
        U2vS = U2S[:].rearrange("p (j r c) -> p j r c", r=2, c=C)
        for xr in range(2):
            for k in range(K):
                Wk = wsv[:, cl, sb, :, xr, k, :]
                if xr == 0:
                    Wb = (
                        Wk.rearrange("p j (r x) -> p j r x", r=2, x=2)
                        .unsqueeze(-1)
                        .broadcast_to([128, J, 2, 2, C])
                    )
                    nc.vector.tensor_tensor(
                        out=UvS, in0=TLx[:, :, 0], in1=Wb, op=M
                    )
                    nc.vector.tensor_tensor(
                        out=U2vS, in0=UvS[:, :, :, 0, :], in1=UvS[:, :, :, 1, :],
                        op=A,
                    )
                else:
                    Wk4 = Wk.rearrange("p j (r x) -> p j r x", r=2, x=2)
                    WA = Wk4[:, :, :, 0].unsqueeze(-1).broadcast_to([128, J, 2, C])
                    WB = Wk4[:, :, :, 1].unsqueeze(-1).broadcast_to([128, J, 2, C])
                    nc.vector.tensor_tensor(
                        out=Uv4S[:, :, 0:2, :], in0=TLx[:, :, 0, :, 1, :], in1=WA,
                        op=M,
                    )
                    nc.vector.tensor_tensor(
                        out=Uv4S[:, :, 2:4, :], in0=TLx[:, :, 1, :, 0, :], in1=WB,
                        op=M,
                    )
                    nc.vector.tensor_tensor(
                        out=U2vS, in0=Uv4S[:, :, 0:2, :], in1=Uv4S[:, :, 2:4, :],
                        op=A,
                    )
                fin = nc.vector.tensor_tensor(
                    out=OSv[:, :, xr, k, :], in0=U2vS[:, :, 0, :],
                    in1=U2vS[:, :, 1, :], op=A,
                )
                if xr == 1 and k == K - 1:
                    fin.then_inc(evS, 1)
        nc.sync.wait_ge(evS, t + 1)
        nc.sync.dma_start(out=outSv[cl, sb], in_=ObS[s2][:]).then_inc(dmaOS, 16)
        nOS += 16
    nc.sync.wait_ge(dmaOS, nOS)

    # ---- leftover gather + blend ----
    nc.gpsimd.wait_ge(dmaP, TOT_STORES)
    nc.gpsimd.wait_ge(dmaW, 48)

    wv = w_t[:].rearrange("p (g j) -> p g j", j=4)
    outv = out_d[:].rearrange("(g p) c -> p g c", p=128)
    M = mybir.AluOpType.mult
    A = mybir.AluOpType.add

    nG = 0
    nO = 0
    bglob = 0
    goff = 0
    for si, (tbl, xr) in enumerate([(cA, 0), (cA, 1), (cB, 0), (cB, 1)]):
        GS = gs[si]
        elen = 128 if xr == 0 else 256
        for b in range(GS // BATCH):
            s = bglob % 2
            if bglob >= 2:
                nc.gpsimd.wait_ge(evB, bglob - 1)
            for gi in range(BATCH):
                g = goff + b * BATCH + gi
                nc.gpsimd.indirect_dma_start(
                    out=Gb[s][:, gi * 256 : gi * 256 + elen],
                    out_offset=None,
                    in_=tbl[:],
                    in_offset=IndirectOffsetOnAxis(
                        ap=idx_t[:, g : g + 1], axis=0
                    ),
                ).then_inc(dmaG, 16)
                nG += 16
            nc.vector.wait_ge(dmaG, nG)
            if bglob >= 2:
                nc.vector.wait_ge(dmaO, 16 * (bglob - 1))
            gsl = slice(goff + b * BATCH, goff + b * BATCH + BATCH)
            # Gb viewed [p, g, cell(2), r(2), x(2), c]
            Gv = Gb[s][:].rearrange(
                "p (g cl r x c) -> p g cl r x c", cl=2, r=2, x=2, c=C
            )
            wv4 = w_t[:].rearrange("p (g r x) -> p g r x", r=2, x=2)
            U2v = U2[:].rearrange("p (g r c) -> p g r c", r=2, c=C)
            Ov = Ob[s][:].rearrange("p (g c) -> p g c", c=C)
            if xr == 0:
                Uv = Ub[:].rearrange("p (g r x c) -> p g r x c", r=2, x=2, c=C)
                Wb = (
                    wv4[:, gsl, :, :]
                    .unsqueeze(-1)
                    .broadcast_to([128, BATCH, 2, 2, C])
                )
                nc.vector.tensor_tensor(out=Uv, in0=Gv[:, :, 0], in1=Wb, op=M)
                nc.vector.tensor_tensor(
                    out=U2v, in0=Uv[:, :, :, 0, :], in1=Uv[:, :, :, 1, :], op=A
                )
            else:
                # x0 pixel = (cell0, x=1) with weight w[:, :, 0]; x0+1 = (cell1, x=0)
                GvA = Gv[:, :, 0, :, 1, :]
                GvB = Gv[:, :, 1, :, 0, :]
                WA = (
                    wv4[:, gsl, :, 0].unsqueeze(-1).broadcast_to([128, BATCH, 2, C])
                )
                WB = (
                    wv4[:, gsl, :, 1].unsqueeze(-1).broadcast_to([128, BATCH, 2, C])
                )
                Uv4 = Ub[:].rearrange("p (g r c) -> p g r c", r=4, c=C)
                T1 = Uv4[:, :, 0:2, :]
                T2 = Uv4[:, :, 2:4, :]
                nc.vector.tensor_tensor(out=T1, in0=GvA, in1=WA, op=M)
                nc.vector.tensor_tensor(out=T2, in0=GvB, in1=WB, op=M)
                nc.vector.tensor_tensor(out=U2v, in0=T1, in1=T2, op=A)
            nc.vector.tensor_tensor(
                out=Ov, in0=U2v[:, :, 0, :], in1=U2v[:, :, 1, :], op=A
            ).then_inc(evB, 1)
            nc.sync.wait_ge(evB, bglob + 1)
            nc.sync.dma_start(out=outv[:, gsl, :], in_=Ob[s][:]).then_inc(dmaO, 16)
            nO += 16
            bglob += 1
        goff += GS
    nc.sync.wait_ge(dmaO, nO)
    return nc


_NC_CACHE = {}


def _get_nc(gs):
    if gs not in _NC_CACHE:
        _NC_CACHE[gs] = _build_program(gs)
    return _NC_CACHE[gs]


def _prep_host(pts, g0, g1, g2, g3):
    f = np.float32
    g0s = np.ascontiguousarray(g0[0].transpose(1, 2, 0)) * f(0.125)
    g3s = np.ascontiguousarray(g3[0].transpose(1, 2, 3, 0)) * f(0.6 * 0.125)
    g2s = np.ascontiguousarray(g2[0].transpose(1, 2, 3, 0)) * f(0.4 * 0.25)
    g1s = np.ascontiguousarray(g1[0].transpose(1, 2, 3, 0)) * f(0.2 * 0.5)

    n = pts.shape[0]
    W1 = f(H - 1)
    x = np.clip((pts[:, 0] + f(1.0)) * f(0.5) * W1, f(0.0), W1)
    y = np.clip((pts[:, 1] + f(1.0)) * f(0.5) * W1, f(0.0), W1)
    x0 = np.floor(x)
    y0 = np.floor(y)
    wx = (x - x0).astype(f)
    wy = (y - y0).astype(f)
    x0i = x0.astype(np.int64)
    y0i = y0.astype(np.int64)
    sx = x0i == H - 1
    x0i = np.where(sx, x0i - 1, x0i)
    wx = np.where(sx, f(1.0), wx)
    sy = y0i == H - 1
    y0i = np.where(sy, y0i - 1, y0i)
    wy = np.where(sy, f(1.0), wy)

    core = (y0i >> 7).astype(np.int32)
    yl = (y0i & 127).astype(np.int32)
    P = yl & 1
    k = yl >> 1
    u = (x0i >> 1).astype(np.int32)
    xr = (x0i & 1).astype(np.int32)
    cell = k * 512 + u
    stream = P * 2 + xr
    w4 = np.stack(
        [(1 - wy) * (1 - wx), (1 - wy) * wx, wy * (1 - wx), wy * wx], axis=1
    ).astype(np.float16)

    order = np.lexsort((cell, stream, core))
    cell_s = cell[order]
    stream_s = stream[order]
    core_s = core[order]
    w4_s = w4[order]
    P_s = stream_s >> 1
    xr_s = stream_s & 1

    # rank within (core, P, cell, xr) bin; first K go to sweep slots
    nn = cell_s.shape[0]
    binid = (((core_s.astype(np.int64) * 2 + P_s) * CELLS + cell_s) * 2 + xr_s)
    newb = np.empty(nn, bool)
    newb[0] = True
    newb[1:] = binid[1:] != binid[:-1]
    first = np.maximum.accumulate(np.where(newb, np.arange(nn), 0))
    rank = (np.arange(nn) - first).astype(np.int32)
    slot = rank < K

    # per (core, stream) leftover counts
    counts = np.zeros((NCORES, 4), np.int64)
    for c in range(NCORES):
        mc = (core_s == c) & ~slot
        for s in range(4):
            counts[c, s] = int(np.sum(mc & (stream_s == s)))
    # SPMD: shared group counts per stream = max over cores, batch-rounded
    gs = tuple(
        max(BATCH,
            int(-(-int(counts[:, s].max()) // (128 * BATCH)) * BATCH))
        for s in range(4)
    )
    GTOT = sum(gs)

    # coefficient slabs per core (zero-padded beyond grid)
    def slab(arr, r0, nr, full):
        if arr.ndim == 4:
            out = np.zeros((3, nr) + arr.shape[2:], np.float16)
            hi = min(full, r0 + nr)
            out[:, : hi - r0] = arr[:, r0:hi].astype(np.float16)
        else:
            out = np.zeros((nr,) + arr.shape[1:], np.float16)
            hi = min(full, r0 + nr)
            out[: hi - r0] = arr[r0:hi].astype(np.float16)
        return out

    in_maps = []
    for c in range(NCORES):
        idx2 = np.zeros((128, GTOT), np.int32)
        wt = np.zeros((128, GTOT, 4), np.float16)
        # sweep slot weights [p, cl, sb, j, xr, k, 4]
        wS = np.zeros((128, 2, NSB, J, 2, K, 4), np.float16)
        msl = (core_s == c) & slot
        ce = cell_s[msl]
        wS[ce & 127, P_s[msl], ce >> 10, (ce >> 7) & 7, xr_s[msl], rank[msl]] = (
            w4_s[msl]
        )
        goff = 0
        for s in range(4):
            sel = (core_s == c) & (stream_s == s) & ~slot
            cells_cs = cell_s[sel]
            w_cs = w4_s[sel]
            cnt = cells_cs.shape[0]
            cap = gs[s] * 128
            assert cnt <= cap, f"stream overflow core {c} stream {s}"
            # point j -> group goff + j//128, partition j%128
            gidx = goff + np.arange(cnt) // 128
            pidx = np.arange(cnt) % 128
            idx2[pidx, gidx] = cells_cs
            wt[pidx, gidx] = w_cs
            goff += gs[s]
        in_maps.append(
            {
                "gA": np.concatenate(
                    [slab(g0s, 16 * c, 18, 128)[None], slab(g3s, 16 * c, 18, 128)]
                ),
                "g2s": slab(g2s, 32 * c, 36, 256),
                "g1s": slab(g1s, 64 * c, 72, 512),
                "idx": idx2,
                "w": np.ascontiguousarray(wt.reshape(128, 4 * GTOT)),
                "ws": np.ascontiguousarray(wS.reshape(128, -1)),
            }
        )
    return in_maps, order, counts, gs, n, (core_s, P_s, xr_s, cell_s, rank, slot)


def kernel(pts, g0, g1, g2, g3, _res_hook=None):
    pts = np.asarray(pts, np.float32)
    in_maps, order, counts, gs, n, meta = _prep_host(
        pts, np.asarray(g0, np.float32), np.asarray(g1, np.float32),
        np.asarray(g2, np.float32), np.asarray(g3, np.float32),
    )
    core_s, P_s, xr_s, cell_s, rank, slot = meta
    nc = _get_nc(gs)
    res = bass_utils.run_bass_kernel_spmd(nc, in_maps, list(range(NCORES)))
    if _res_hook is not None:
        _res_hook(res)
    out_sorted = np.empty((n, C), np.float32)
    for c in range(NCORES):
        mc = core_s == c
        # sweep-slotted points
        oS = res.results[c]["outS"].reshape(2, CELLS, 2, K, C)
        msl = mc & slot
        out_sorted[msl] = oS[
            P_s[msl], cell_s[msl], xr_s[msl], rank[msl]
        ].astype(np.float32)
        # leftover points, packed per stream in sorted order
        o = res.results[c]["out"]
        goff = 0
        for s in range(4):
            sel = mc & ~slot & ((P_s * 2 + xr_s) == s)
            cnt = int(counts[c, s])
            out_sorted[sel] = o[goff * 128 : goff * 128 + cnt].astype(np.float32)
            goff += gs[s]
    full = np.empty_like(out_sorted)
    full[order] = out_sorted
    return full


# revision 20
# speedup vs baseline: 5.5970x; 1.0540x over previous
import sys

sys.path.insert(0, "/opt/trn_rl_repo")
import numpy as np
from concourse import mybir
from concourse.bass import Bass, IndirectOffsetOnAxis
from concourse import bass_utils

C = 32
H = 1024
NCORES = 8
BATCH = 16            # groups per blend batch
CELLS = 64 * 512      # pair-cells per class table per core
SLAB = 144            # local plane rows computed per core (18 coarse rows)
K = 3                 # capacity slots per (cell, xr) in the sweep
J = 16                # cell-blocks (of 128 cells) per sweep iteration
NSB = CELLS // (J * 128)   # sweep iterations per class table

_f16 = mybir.dt.float16
_i32 = mybir.dt.int32


def _build_program(gs):
    """gs: tuple of 4 group counts (classA/xr0, classA/xr1, classB/xr0, classB/xr1),
    each a multiple of BATCH. One SPMD program; per-core data via in_maps."""
    GTOT = sum(gs)
    nc = Bass()
    gA = nc.declare_dram_parameter("gA", [4, 18, 128, C], _f16, isOutput=False)
    ll2b = nc.declare_dram_parameter("ll2b", [4, 36, 256, C], _f16, isOutput=False)
    ll1b = nc.declare_dram_parameter("ll1b", [4, 72, 512, C], _f16, isOutput=False)
    idx_d = nc.declare_dram_parameter("idx", [128, GTOT], _i32, isOutput=False)
    w_d = nc.declare_dram_parameter("w", [128, 4 * GTOT], _f16, isOutput=False)
    # sweep slot weights: [p, cl, sb, j, xr, k, 4]
    WTOT = 2 * NSB * J * 2 * K * 4
    ws_d = nc.declare_dram_parameter("ws", [128, WTOT], _f16, isOutput=False)
    out_d = nc.declare_dram_parameter("out", [GTOT * 128, C], _f16, isOutput=True)
    outS_d = nc.declare_dram_parameter(
        "outS", [2 * CELLS, 2 * K * C], _f16, isOutput=True
    )

    cA = nc.dram_tensor("cA", [CELLS + 1, 2 * 2 * C], _f16)  # [k*512+u, (r,x,c)]
    cB = nc.dram_tensor("cB", [CELLS + 1, 2 * 2 * C], _f16)

    dmaL = nc.alloc_semaphore()   # idwt input loads
    dmaP = nc.alloc_semaphore()   # idwt stores
    evW = nc.alloc_semaphore()    # idwt compute iters
    dmaW = nc.alloc_semaphore()   # idx/w loads
    dmaG = nc.alloc_semaphore()   # indirect gathers (must start at 0)
    evB = nc.alloc_semaphore()    # blend batches
    dmaO = nc.alloc_semaphore()   # output stores
    dmaS = nc.alloc_semaphore()   # sweep table loads
    evS = nc.alloc_semaphore()    # sweep blend iterations
    dmaOS = nc.alloc_semaphore()  # sweep output stores

    # ---- SBUF ----
    CHW = 64 * C                  # 64 output-col-pairs worth of one parity = 2048 els
    inb = [nc.alloc_sbuf_tensor(f"in{s}", [128, 4 * CHW], _f16).ap()
           for s in range(2)]
    tmp = [[nc.alloc_sbuf_tensor(f"tmp{s}_{k}", [128, CHW], _f16).ap()
            for k in range(4)] for s in range(2)]
    # out bigbuf layout [p, w(64), rp(2), xp(2), c]
    outb = [nc.alloc_sbuf_tensor(f"out{s}", [128, 4 * CHW], _f16).ap()
            for s in range(2)]
    idx_t = nc.alloc_sbuf_tensor("idx_t", [128, GTOT], _i32).ap()
    w_t = nc.alloc_sbuf_tensor("w_t", [128, 4 * GTOT], _f16).ap()
    Gb = [nc.alloc_sbuf_tensor(f"G{s}", [128, BATCH * 256], _f16).ap() for s in range(2)]
    Ub = nc.alloc_sbuf_tensor("Ub", [128, BATCH * 4 * C], _f16).ap()
    U2 = nc.alloc_sbuf_tensor("U2", [128, BATCH * 2 * C], _f16).ap()
    Ob = [nc.alloc_sbuf_tensor(f"O{s}", [128, BATCH * C], _f16).ap() for s in range(2)]
    ws_t = nc.alloc_sbuf_tensor("ws_t", [128, WTOT], _f16).ap()
    TL = [nc.alloc_sbuf_tensor(f"TL{s}", [128, J * 2 * 128], _f16).ap()
          for s in range(2)]
    UbS = nc.alloc_sbuf_tensor("UbS", [128, J * 4 * C], _f16).ap()
    U2S = nc.alloc_sbuf_tensor("U2S", [128, J * 2 * C], _f16).ap()
    ObS = [nc.alloc_sbuf_tensor(f"OS{s}", [128, J * 2 * K * C], _f16).ap()
           for s in range(2)]

    # ---- IDWT ----
    # levels: A: 18 coarse rows,128 wide -> ll2 (36,256); 2 col chunks of 64
    #         B: 36 rows,256 wide -> ll1 (72,512); 4 chunks
    #         C: 72 rows,512 wide -> cA/cB tables; 8 chunks
    iters = [("A", cb) for cb in range(2)] + [("B", cb) for cb in range(4)] + \
            [("C", cb) for cb in range(8)]
    NIT = len(iters)
    NPL = {"A": 18, "B": 36, "C": 72}
    ll2v = ll2b[0].rearrange("(p two) (w xp) c -> p two w xp c", two=2, xp=2)
    ll1v = ll1b[0].rearrange("(p two) (w xp) c -> p two w xp c", two=2, xp=2)
    cAv = cA[0:CELLS].rearrange("(k u) (r x c) -> k u r x c", u=512, r=2, x=2)
    cBv = cB[0:CELLS].rearrange("(k u) (r x c) -> k u r x c", u=512, r=2, x=2)

    def src_packed(level, cb):
        cs = slice(cb * 64, cb * 64 + 64)
        if level == "A":
            return gA[:, :, cs, :].rearrange("b p w c -> p b w c")
        if level == "B":
            return ll2b[:, :, cs, :].rearrange("b p w c -> p b w c")
        return ll1b[:, :, cs, :].rearrange("b p w c -> p b w c")

    stores_per_iter = {"A": 4, "B": 4, "C": 3}
    cum_stores = []
    tot = 0
    for lv, _ in iters:
        cum_stores.append(tot)
        tot += stores_per_iter[lv] * 16
    TOT_STORES = tot
    lvlB_start, lvlC_start = 2, 6

    nL = 0
    nW = 0
    for i, (level, cb) in enumerate(iters):
        s = i % 2
        NP = NPL[level]
        if i >= 2:
            nc.sync.wait_ge(evW, i - 1)
        if i == lvlB_start:
            nc.sync.wait_ge(dmaP, cum_stores[lvlB_start])
        if i == lvlC_start:
            nc.sync.wait_ge(dmaP, cum_stores[lvlC_start])
        nc.sync.dma_start(
            out=inb[s][:NP].rearrange("p (b w c) -> p b w c", b=4, c=C),
            in_=src_packed(level, cb),
        ).then_inc(dmaL, 16)
        nL += 16
        nc.vector.wait_ge(dmaL, nL)
        if i >= 2:
            nc.vector.wait_ge(dmaP, cum_stores[i - 1])  # stores of iter i-2 done
        A = mybir.AluOpType.add
        S = mybir.AluOpType.subtract
        inv = inb[s][:NP].rearrange("p (b e) -> p b e", b=4)
        ll, lh, hl, hh = (inv[:, k] for k in range(4))
        t1, t2, t3, t4 = (b[:NP] for b in tmp[s])
        ov = outb[s][:NP].rearrange("p (w rp xp c) -> p w rp xp c", rp=2, xp=2, c=C)
        oEE = ov[:, :, 0, 0, :]
        oEO = ov[:, :, 0, 1, :]
        oOE = ov[:, :, 1, 0, :]
        oOO = ov[:, :, 1, 1, :]
        nc.vector.tensor_tensor(out=t1, in0=ll, in1=lh, op=S)   # row-even lo
        nc.vector.tensor_tensor(out=t2, in0=ll, in1=lh, op=A)   # row-odd lo
        nc.vector.tensor_tensor(out=t3, in0=hl, in1=hh, op=S)   # row-even hi
        nc.vector.tensor_tensor(out=t4, in0=hl, in1=hh, op=A)   # row-odd hi
        nc.vector.tensor_tensor(out=oEE, in0=t1, in1=t3, op=S)  # (2r, 2w)
        nc.vector.tensor_tensor(out=oEO, in0=t1, in1=t3, op=A)  # (2r, 2w+1)
        nc.vector.tensor_tensor(out=oOE, in0=t2, in1=t4, op=S)  # (2r+1, 2w)
        nc.vector.tensor_tensor(out=oOO, in0=t2, in1=t4, op=A).then_inc(evW, 1)
        nW += 1
        nc.sync.wait_ge(evW, nW)
        ws = slice(cb * 64, cb * 64 + 64)
        bufv = outb[s][:].rearrange("p (w rp xp c) -> p w rp xp c", rp=2, xp=2, c=C)
        if level in ("A", "B"):
            dstv = ll2v if level == "A" else ll1v
            for rp in range(2):
                for xp in range(2):
                    nc.sync.dma_start(
                        out=dstv[:NP, rp, ws, xp, :],
                        in_=bufv[:NP, :, rp, xp, :],
                    ).then_inc(dmaP, 16)
        else:
            # classA: pair k=p rows (2p, 2p+1): full cells, contiguous
            nc.sync.dma_start(
                out=cAv[0:64, ws, :, :, :], in_=bufv[0:64]
            ).then_inc(dmaP, 16)
            # classB r0 = odd rows (2p+1): k=p
            nc.sync.dma_start(
                out=cBv[0:64, ws, 0, :, :], in_=bufv[0:64, :, 1, :, :]
            ).then_inc(dmaP, 16)
            # classB r1 = even rows (2p), p=1..64 -> k=p-1
            nc.sync.dma_start(
                out=cBv[0:64, ws, 1, :, :], in_=bufv[1:65, :, 0, :, :]
            ).then_inc(dmaP, 16)

    # ---- sweep: K slots per (cell, xr), plain DMA loads, static-AP blends ----
    nc.sync.dma_start(out=idx_t[:], in_=idx_d[:]).then_inc(dmaW, 16)
    nc.sync.dma_start(out=w_t[:], in_=w_d[:]).then_inc(dmaW, 16)
    nc.sync.dma_start(out=ws_t[:], in_=ws_d[:]).then_inc(dmaW, 16)
    M = mybir.AluOpType.mult
    A = mybir.AluOpType.add
    wsv = ws_t[:].rearrange(
        "p (cl sb j xr k q) -> p cl sb j xr k q", cl=2, sb=NSB, j=J, xr=2, k=K
    )
    outSv = outS_d[:].rearrange(
        "(cl sb j p) w -> cl sb p j w", cl=2, sb=NSB, j=J, p=128
    )
    nc.scalar.wait_ge(dmaP, TOT_STORES)   # tables complete before sweep loads
    nc.vector.wait_ge(dmaW, 48)
    nOS = 0
    for t in range(2 * NSB):
        cl, sb = t // NSB, t % NSB
        tbl = cA if cl == 0 else cB
        own = tbl[0:CELLS].rearrange("(sb j p) e -> sb p j e", sb=NSB, j=J, p=128)
        nxt = tbl[1 : CELLS + 1].rearrange(
            "(sb j p) e -> sb p j e", sb=NSB, j=J, p=128
        )
        s2 = t % 2
        TLv = TL[s2][:].rearrange("p (j h e) -> p j h e", h=2, e=128)
        if t >= 2:
            nc.scalar.wait_ge(evS, t - 1)
        nc.scalar.dma_start(out=TLv[:, :, 0, :], in_=own[sb]).then_inc(dmaS, 16)
        nc.scalar.dma_start(out=TLv[:, :, 1, :], in_=nxt[sb]).then_inc(dmaS, 16)
        nc.vector.wait_ge(dmaS, 32 * (t + 1))
        if t >= 2:
            nc.vector.wait_ge(dmaOS, 16 * (t - 1))
        TLx = TL[s2][:].rearrange(
            "p (j h r x c) -> p j h r x c", h=2, r=2, x=2, c=C
        )
        OSv = ObS[s2][:].rearrange("p (j xr k c) -> p j xr k c", xr=2, k=K, c=C)
        UvS = UbS[:].rearrange("p (j r x c) -> p j r x c", r=2, x=2, c=C)
        Uv4S = UbS[:].rearrange("p (j r c) -> p j r c", r=4, c=C)
        U2vS = U2S[:].rearrange("p (j r c) -> p j r c", r=2, c=C)
        for xr in range(2):
            for k in range(K):
                Wk = wsv[:, cl, sb, :, xr, k, :]
                if xr == 0:
                    Wb = (
                        Wk.rearrange("p j (r x) -> p j r x", r=2, x=2)
                        .unsqueeze(-1)
                        .broadcast_to([128, J, 2, 2, C])
                    )
                    nc.vector.tensor_tensor(
                        out=UvS, in0=TLx[:, :, 0], in1=Wb, op=M
                    )
                    nc.vector.tensor_tensor(
                        out=U2vS, in0=UvS[:, :, :, 0, :], in1=UvS[:, :, :, 1, :],
                        op=A,
                    )
                else:
                    Wk4 = Wk.rearrange("p j (r x) -> p j r x", r=2, x=2)
                    WA = Wk4[:, :, :, 0].unsqueeze(-1).broadcast_to([128, J, 2, C])
                    WB = Wk4[:, :, :, 1].unsqueeze(-1).broadcast_to([128, J, 2, C])
                    nc.vector.tensor_tensor(
                        out=Uv4S[:, :, 0:2, :], in0=TLx[:, :, 0, :, 1, :], in1=WA,
                        op=M,
                    )
                    nc.vector.tensor_tensor(
                        out=Uv4S[:, :, 2:4, :], in0=TLx[:, :, 1, :, 0, :], in1=WB,
                        op=M,
                    )
                    nc.vector.tensor_tensor(
                        out=U2vS, in0=Uv4S[:, :, 0:2, :], in1=Uv4S[:, :, 2:4, :],
                        op=A,
                    )
                fin = nc.vector.tensor_tensor(
                    out=OSv[:, :, xr, k, :], in0=U2vS[:, :, 0, :],
                    in1=U2vS[:, :, 1, :], op=A,
                )
                if xr == 1 and k == K - 1:
                    fin.then_inc(evS, 1)
        nc.sync.wait_ge(evS, t + 1)
        nc.sync.dma_start(out=outSv[cl, sb], in_=ObS[s2][:]).then_inc(dmaOS, 16)
        nOS += 16
    nc.sync.wait_ge(dmaOS, nOS)

    # ---- leftover gather + blend ----
    nc.gpsimd.wait_ge(dmaP, TOT_STORES)
    nc.gpsimd.wait_ge(dmaW, 48)

    wv = w_t[:].rearrange("p (g j) -> p g j", j=4)
    outv = out_d[:].rearrange("(g p) c -> p g c", p=128)
    M = mybir.AluOpType.mult
    A = mybir.AluOpType.add

    nG = 0
    nO = 0
    bglob = 0
    goff = 0
    for si, (tbl, xr) in enumerate([(cA, 0), (cA, 1), (cB, 0), (cB, 1)]):
        GS = gs[si]
        elen = 128 if xr == 0 else 256
        for b in range(GS // BATCH):
            s = bglob % 2
            if bglob >= 2:
                nc.gpsimd.wait_ge(evB, bglob - 1)
            for gi in range(BATCH):
                g = goff + b * BATCH + gi
                nc.gpsimd.indirect_dma_start(
                    out=Gb[s][:, gi * 256 : gi * 256 + elen],
                    out_offset=None,
                    in_=tbl[:],
                    in_offset=IndirectOffsetOnAxis(
                        ap=idx_t[:, g : g + 1], axis=0
                    ),
                ).then_inc(dmaG, 16)
                nG += 16
            nc.vector.wait_ge(dmaG, nG)
            if bglob >= 2:
                nc.vector.wait_ge(dmaO, 16 * (bglob - 1))
            gsl = slice(goff + b * BATCH, goff + b * BATCH + BATCH)
            # Gb viewed [p, g, cell(2), r(2), x(2), c]
            Gv = Gb[s][:].rearrange(
                "p (g cl r x c) -> p g cl r x c", cl=2, r=2, x=2, c=C
            )
            wv4 = w_t[:].rearrange("p (g r x) -> p g r x", r=2, x=2)
            U2v = U2[:].rearrange("p (g r c) -> p g r c", r=2, c=C)
            Ov = Ob[s][:].rearrange("p (g c) -> p g c", c=C)
            if xr == 0:
                Uv = Ub[:].rearrange("p (g r x c) -> p g r x c", r=2, x=2, c=C)
                Wb = (
                    wv4[:, gsl, :, :]
                    .unsqueeze(-1)
                    .broadcast_to([128, BATCH, 2, 2, C])
                )
                nc.vector.tensor_tensor(out=Uv, in0=Gv[:, :, 0], in1=Wb, op=M)
                nc.vector.tensor_tensor(
                    out=U2v, in0=Uv[:, :, :, 0, :], in1=Uv[:, :, :, 1, :], op=A
                )
            else:
                # x0 pixel = (cell0, x=1) with weight w[:, :, 0]; x0+1 = (cell1, x=0)
                GvA = Gv[:, :, 0, :, 1, :]
                GvB = Gv[:, :, 1, :, 0, :]
                WA = (
                    wv4[:, gsl, :, 0].unsqueeze(-1).broadcast_to([128, BATCH, 2, C])
                )
                WB = (
                    wv4[:, gsl, :, 1].unsqueeze(-1).broadcast_to([128, BATCH, 2, C])
                )
                Uv4 = Ub[:].rearrange("p (g r c) -> p g r c", r=4, c=C)
                T1 = Uv4[:, :, 0:2, :]
                T2 = Uv4[:, :, 2:4, :]
                nc.vector.tensor_tensor(out=T1, in0=GvA, in1=WA, op=M)
                nc.vector.tensor_tensor(out=T2, in0=GvB, in1=WB, op=M)
                nc.vector.tensor_tensor(out=U2v, in0=T1, in1=T2, op=A)
            nc.vector.tensor_tensor(
                out=Ov, in0=U2v[:, :, 0, :], in1=U2v[:, :, 1, :], op=A
            ).then_inc(evB, 1)
            nc.sync.wait_ge(evB, bglob + 1)
            nc.sync.dma_start(out=outv[:, gsl, :], in_=Ob[s][:]).then_inc(dmaO, 16)
            nO += 16
            bglob += 1
        goff += GS
    nc.sync.wait_ge(dmaO, nO)
    return nc


_NC_CACHE = {}


def _get_nc(gs):
    if gs not in _NC_CACHE:
        _NC_CACHE[gs] = _build_program(gs)
    return _NC_CACHE[gs]


def _prep_host(pts, g0, g1, g2, g3):
    f = np.float32
    g0s = np.ascontiguousarray(g0[0].transpose(1, 2, 0)) * f(0.125)
    g3s = np.ascontiguousarray(g3[0].transpose(1, 2, 3, 0)) * f(0.6 * 0.125)
    g2s = np.ascontiguousarray(g2[0].transpose(1, 2, 3, 0)) * f(0.4 * 0.25)
    g1s = np.ascontiguousarray(g1[0].transpose(1, 2, 3, 0)) * f(0.2 * 0.5)

    n = pts.shape[0]
    W1 = f(H - 1)
    x = np.clip((pts[:, 0] + f(1.0)) * f(0.5) * W1, f(0.0), W1)
    y = np.clip((pts[:, 1] + f(1.0)) * f(0.5) * W1, f(0.0), W1)
    x0 = np.floor(x)
    y0 = np.floor(y)
    wx = (x - x0).astype(f)
    wy = (y - y0).astype(f)
    x0i = x0.astype(np.int64)
    y0i = y0.astype(np.int64)
    sx = x0i == H - 1
    x0i = np.where(sx, x0i - 1, x0i)
    wx = np.where(sx, f(1.0), wx)
    sy = y0i == H - 1
    y0i = np.where(sy, y0i - 1, y0i)
    wy = np.where(sy, f(1.0), wy)

    core = (y0i >> 7).astype(np.int32)
    yl = (y0i & 127).astype(np.int32)
    P = yl & 1
    k = yl >> 1
    u = (x0i >> 1).astype(np.int32)
    xr = (x0i & 1).astype(np.int32)
    cell = k * 512 + u
    stream = P * 2 + xr
    w4 = np.stack(
        [(1 - wy) * (1 - wx), (1 - wy) * wx, wy * (1 - wx), wy * wx], axis=1
    ).astype(np.float16)

    order = np.lexsort((cell, stream, core))
    cell_s = cell[order]
    stream_s = stream[order]
    core_s = core[order]
    w4_s = w4[order]
    P_s = stream_s >> 1
    xr_s = stream_s & 1

    # rank within (core, P, cell, xr) bin; first K go to sweep slots
    nn = cell_s.shape[0]
    binid = (((core_s.astype(np.int64) * 2 + P_s) * CELLS + cell_s) * 2 + xr_s)
    newb = np.empty(nn, bool)
    newb[0] = True
    newb[1:] = binid[1:] != binid[:-1]
    first = np.maximum.accumulate(np.where(newb, np.arange(nn), 0))
    rank = (np.arange(nn) - first).astype(np.int32)
    slot = rank < K

    # per (core, stream) leftover counts
    counts = np.zeros((NCORES, 4), np.int64)
    for c in range(NCORES):
        mc = (core_s == c) & ~slot
        for s in range(4):
            counts[c, s] = int(np.sum(mc & (stream_s == s)))
    # SPMD: shared group counts per stream = max over cores, batch-rounded
    gs = tuple(
        max(BATCH,
            int(-(-int(counts[:, s].max()) // (128 * BATCH)) * BATCH))
        for s in range(4)
    )
    GTOT = sum(gs)

    # coefficient slabs per core (zero-padded beyond grid)
    def slab(arr, r0, nr, full):
        if arr.ndim == 4:
            out = np.zeros((3, nr) + arr.shape[2:], np.float16)
            hi = min(full, r0 + nr)
            out[:, : hi - r0] = arr[:, r0:hi].astype(np.float16)
        else:
            out = np.zeros((nr,) + arr.shape[1:], np.float16)
            hi = min(full, r0 + nr)
            out[: hi - r0] = arr[r0:hi].astype(np.float16)
        return out

    in_maps = []
    for c in range(NCORES):
        idx2 = np.zeros((128, GTOT), np.int32)
        wt = np.zeros((128, GTOT, 4), np.float16)
        # sweep slot weights [p, cl, sb, j, xr, k, 4]
        wS = np.zeros((128, 2, NSB, J, 2, K, 4), np.float16)
        msl = (core_s == c) & slot
        ce = cell_s[msl]
        wS[ce & 127, P_s[msl], ce // (J * 128), (ce >> 7) % J, xr_s[msl],
           rank[msl]] = w4_s[msl]
        goff = 0
        for s in range(4):
            sel = (core_s == c) & (stream_s == s) & ~slot
            cells_cs = cell_s[sel]
            w_cs = w4_s[sel]
            cnt = cells_cs.shape[0]
            cap = gs[s] * 128
            assert cnt <= cap, f"stream overflow core {c} stream {s}"
            # point j -> group goff + j//128, partition j%128
            gidx = goff + np.arange(cnt) // 128
            pidx = np.arange(cnt) % 128
            idx2[pidx, gidx] = cells_cs
            wt[pidx, gidx] = w_cs
            goff += gs[s]
        in_maps.append(
            {
                "gA": np.concatenate(
                    [slab(g0s, 16 * c, 18, 128)[None], slab(g3s, 16 * c, 18, 128)]
                ),
                "ll2b": np.concatenate(
                    [np.zeros((1, 36, 256, C), np.float16),
                     slab(g2s, 32 * c, 36, 256)]
                ),
                "ll1b": np.concatenate(
                    [np.zeros((1, 72, 512, C), np.float16),
                     slab(g1s, 64 * c, 72, 512)]
                ),
                "idx": idx2,
                "w": np.ascontiguousarray(wt.reshape(128, 4 * GTOT)),
                "ws": np.ascontiguousarray(wS.reshape(128, -1)),
            }
        )
    return in_maps, order, counts, gs, n, (core_s, P_s, xr_s, cell_s, rank, slot)


def kernel(pts, g0, g1, g2, g3, _res_hook=None):
    pts = np.asarray(pts, np.float32)
    in_maps, order, counts, gs, n, meta = _prep_host(
        pts, np.asarray(g0, np.float32), np.asarray(g1, np.float32),
        np.asarray(g2, np.float32), np.asarray(g3, np.float32),
    )
    core_s, P_s, xr_s, cell_s, rank, slot = meta
    nc = _get_nc(gs)
    res = bass_utils.run_bass_kernel_spmd(nc, in_maps, list(range(NCORES)))
    if _res_hook is not None:
        _res_hook(res)
    out_sorted = np.empty((n, C), np.float32)
    for c in range(NCORES):
        mc = core_s == c
        # sweep-slotted points
        oS = res.results[c]["outS"].reshape(2, CELLS, 2, K, C)
        msl = mc & slot
        out_sorted[msl] = oS[
            P_s[msl], cell_s[msl], xr_s[msl], rank[msl]
        ].astype(np.float32)
        # leftover points, packed per stream in sorted order
        o = res.results[c]["out"]
        goff = 0
        for s in range(4):
            sel = mc & ~slot & ((P_s * 2 + xr_s) == s)
            cnt = int(counts[c, s])
            out_sorted[sel] = o[goff * 128 : goff * 128 + cnt].astype(np.float32)
            goff += gs[s]
    full = np.empty_like(out_sorted)
    full[order] = out_sorted
    return full


# revision 21
# speedup vs baseline: 6.3411x; 1.1329x over previous
import sys

sys.path.insert(0, "/opt/trn_rl_repo")
import numpy as np
from concourse import mybir
from concourse.bass import Bass, IndirectOffsetOnAxis
from concourse import bass_utils

C = 32
H = 1024
NCORES = 8
BATCH = 16            # groups per blend batch
CELLS = 64 * 512      # pair-cells per class table per core
SLAB = 144            # local plane rows computed per core (18 coarse rows)
K = 3                 # capacity slots per (cell, xr) in the sweep
J = 32                # cell-blocks (of 128 cells) per sweep iteration
NSB = CELLS // (J * 128)   # sweep iterations per class table

_f16 = mybir.dt.float16
_i32 = mybir.dt.int32


def _build_program(gs):
    """gs: tuple of 4 group counts (classA/xr0, classA/xr1, classB/xr0, classB/xr1),
    each a multiple of BATCH. One SPMD program; per-core data via in_maps."""
    GTOT = sum(gs)
    nc = Bass()
    gA = nc.declare_dram_parameter("gA", [4, 18, 128, C], _f16, isOutput=False)
    ll2b = nc.declare_dram_parameter("ll2b", [4, 36, 256, C], _f16, isOutput=False)
    ll1b = nc.declare_dram_parameter("ll1b", [4, 72, 512, C], _f16, isOutput=False)
    idx_d = nc.declare_dram_parameter("idx", [128, GTOT], _i32, isOutput=False)
    w_d = nc.declare_dram_parameter("w", [128, 4 * GTOT], _f16, isOutput=False)
    # sweep slot weights: [p, cl, sb, j, xr, k, 4]
    WTOT = 2 * NSB * J * 2 * K * 4
    ws_d = nc.declare_dram_parameter("ws", [128, WTOT], _f16, isOutput=False)
    out_d = nc.declare_dram_parameter("out", [GTOT * 128, C], _f16, isOutput=True)
    outS_d = nc.declare_dram_parameter(
        "outS", [2 * CELLS, 2 * K * C], _f16, isOutput=True
    )

    cA = nc.dram_tensor("cA", [CELLS + 1, 2 * 2 * C], _f16)  # [k*512+u, (r,x,c)]
    cB = nc.dram_tensor("cB", [CELLS + 1, 2 * 2 * C], _f16)

    dmaL = nc.alloc_semaphore()   # idwt input loads
    dmaP = nc.alloc_semaphore()   # idwt stores
    evW = nc.alloc_semaphore()    # idwt compute iters
    dmaW = nc.alloc_semaphore()   # idx/w loads
    dmaG = nc.alloc_semaphore()   # indirect gathers (must start at 0)
    evB = nc.alloc_semaphore()    # blend batches
    dmaO = nc.alloc_semaphore()   # output stores
    dmaS = nc.alloc_semaphore()   # sweep table loads
    evS = nc.alloc_semaphore()    # sweep blend iterations
    dmaOS = nc.alloc_semaphore()  # sweep output stores

    # ---- SBUF ----
    CHW = 64 * C                  # 64 output-col-pairs worth of one parity = 2048 els
    inb = [nc.alloc_sbuf_tensor(f"in{s}", [128, 4 * CHW], _f16).ap()
           for s in range(2)]
    tmp = [[nc.alloc_sbuf_tensor(f"tmp{s}_{k}", [128, CHW], _f16).ap()
            for k in range(4)] for s in range(2)]
    # out bigbuf layout [p, w(64), rp(2), xp(2), c]
    outb = [nc.alloc_sbuf_tensor(f"out{s}", [128, 4 * CHW], _f16).ap()
            for s in range(2)]
    idx_t = nc.alloc_sbuf_tensor("idx_t", [128, GTOT], _i32).ap()
    w_t = nc.alloc_sbuf_tensor("w_t", [128, 4 * GTOT], _f16).ap()
    Gb = [nc.alloc_sbuf_tensor(f"G{s}", [128, BATCH * 256], _f16).ap() for s in range(2)]
    Ub = nc.alloc_sbuf_tensor("Ub", [128, BATCH * 4 * C], _f16).ap()
    U2 = nc.alloc_sbuf_tensor("U2", [128, BATCH * 2 * C], _f16).ap()
    Ob = [nc.alloc_sbuf_tensor(f"O{s}", [128, BATCH * C], _f16).ap() for s in range(2)]
    ws_t = nc.alloc_sbuf_tensor("ws_t", [128, WTOT], _f16).ap()
    TL = [inb[0], outb[0]]  # reuse idwt buffers (idle once tables are stored)
    UbS = nc.alloc_sbuf_tensor("UbS", [128, J * 4 * C], _f16).ap()
    U2S = nc.alloc_sbuf_tensor("U2S", [128, J * 2 * C], _f16).ap()
    ObS = [nc.alloc_sbuf_tensor(f"OS{s}", [128, J * 2 * K * C], _f16).ap()
           for s in range(2)]

    # ---- IDWT ----
    # levels: A: 18 coarse rows,128 wide -> ll2 (36,256); 2 col chunks of 64
    #         B: 36 rows,256 wide -> ll1 (72,512); 4 chunks
    #         C: 72 rows,512 wide -> cA/cB tables; 8 chunks
    iters = [("A", cb) for cb in range(2)] + [("B", cb) for cb in range(4)] + \
            [("C", cb) for cb in range(8)]
    NIT = len(iters)
    NPL = {"A": 18, "B": 36, "C": 72}
    ll2v = ll2b[0].rearrange("(p two) (w xp) c -> p two w xp c", two=2, xp=2)
    ll1v = ll1b[0].rearrange("(p two) (w xp) c -> p two w xp c", two=2, xp=2)
    cAv = cA[0:CELLS].rearrange("(k u) (r x c) -> k u r x c", u=512, r=2, x=2)
    cBv = cB[0:CELLS].rearrange("(k u) (r x c) -> k u r x c", u=512, r=2, x=2)

    def src_packed(level, cb):
        cs = slice(cb * 64, cb * 64 + 64)
        if level == "A":
            return gA[:, :, cs, :].rearrange("b p w c -> p b w c")
        if level == "B":
            return ll2b[:, :, cs, :].rearrange("b p w c -> p b w c")
        return ll1b[:, :, cs, :].rearrange("b p w c -> p b w c")

    stores_per_iter = {"A": 4, "B": 4, "C": 3}
    cum_stores = []
    tot = 0
    for lv, _ in iters:
        cum_stores.append(tot)
        tot += stores_per_iter[lv] * 16
    TOT_STORES = tot
    lvlB_start, lvlC_start = 2, 6

    nL = 0
    nW = 0
    for i, (level, cb) in enumerate(iters):
        s = i % 2
        NP = NPL[level]
        if i >= 2:
            nc.sync.wait_ge(evW, i - 1)
        if i == lvlB_start:
            nc.sync.wait_ge(dmaP, cum_stores[lvlB_start])
        if i == lvlC_start:
            nc.sync.wait_ge(dmaP, cum_stores[lvlC_start])
        nc.sync.dma_start(
            out=inb[s][:NP].rearrange("p (b w c) -> p b w c", b=4, c=C),
            in_=src_packed(level, cb),
        ).then_inc(dmaL, 16)
        nL += 16
        nc.vector.wait_ge(dmaL, nL)
        if i >= 2:
            nc.vector.wait_ge(dmaP, cum_stores[i - 1])  # stores of iter i-2 done
        A = mybir.AluOpType.add
        S = mybir.AluOpType.subtract
        inv = inb[s][:NP].rearrange("p (b e) -> p b e", b=4)
        ll, lh, hl, hh = (inv[:, k] for k in range(4))
        t1, t2, t3, t4 = (b[:NP] for b in tmp[s])
        ov = outb[s][:NP].rearrange("p (w rp xp c) -> p w rp xp c", rp=2, xp=2, c=C)
        oEE = ov[:, :, 0, 0, :]
        oEO = ov[:, :, 0, 1, :]
        oOE = ov[:, :, 1, 0, :]
        oOO = ov[:, :, 1, 1, :]
        nc.vector.tensor_tensor(out=t1, in0=ll, in1=lh, op=S)   # row-even lo
        nc.vector.tensor_tensor(out=t2, in0=ll, in1=lh, op=A)   # row-odd lo
        nc.vector.tensor_tensor(out=t3, in0=hl, in1=hh, op=S)   # row-even hi
        nc.vector.tensor_tensor(out=t4, in0=hl, in1=hh, op=A)   # row-odd hi
        nc.vector.tensor_tensor(out=oEE, in0=t1, in1=t3, op=S)  # (2r, 2w)
        nc.vector.tensor_tensor(out=oEO, in0=t1, in1=t3, op=A)  # (2r, 2w+1)
        nc.vector.tensor_tensor(out=oOE, in0=t2, in1=t4, op=S)  # (2r+1, 2w)
        nc.vector.tensor_tensor(out=oOO, in0=t2, in1=t4, op=A).then_inc(evW, 1)
        nW += 1
        nc.scalar.wait_ge(evW, nW)
        ws = slice(cb * 64, cb * 64 + 64)
        bufv = outb[s][:].rearrange("p (w rp xp c) -> p w rp xp c", rp=2, xp=2, c=C)
        if level in ("A", "B"):
            dstv = ll2v if level == "A" else ll1v
            for rp in range(2):
                for xp in range(2):
                    nc.scalar.dma_start(
                        out=dstv[:NP, rp, ws, xp, :],
                        in_=bufv[:NP, :, rp, xp, :],
                    ).then_inc(dmaP, 16)
        else:
            # classA: pair k=p rows (2p, 2p+1): full cells, contiguous
            nc.scalar.dma_start(
                out=cAv[0:64, ws, :, :, :], in_=bufv[0:64]
            ).then_inc(dmaP, 16)
            # classB r0 = odd rows (2p+1): k=p
            nc.scalar.dma_start(
                out=cBv[0:64, ws, 0, :, :], in_=bufv[0:64, :, 1, :, :]
            ).then_inc(dmaP, 16)
            # classB r1 = even rows (2p), p=1..64 -> k=p-1
            nc.scalar.dma_start(
                out=cBv[0:64, ws, 1, :, :], in_=bufv[1:65, :, 0, :, :]
            ).then_inc(dmaP, 16)

    # ---- sweep: K slots per (cell, xr), plain DMA loads, static-AP blends ----
    nc.sync.dma_start(out=idx_t[:], in_=idx_d[:]).then_inc(dmaW, 16)
    nc.sync.dma_start(out=w_t[:], in_=w_d[:]).then_inc(dmaW, 16)
    nc.sync.dma_start(out=ws_t[:], in_=ws_d[:]).then_inc(dmaW, 16)
    M = mybir.AluOpType.mult
    A = mybir.AluOpType.add
    wsv = ws_t[:].rearrange(
        "p (cl sb j xr k q) -> p cl sb j xr k q", cl=2, sb=NSB, j=J, xr=2, k=K
    )
    outSv = outS_d[:].rearrange(
        "(cl sb j p) w -> cl sb p j w", cl=2, sb=NSB, j=J, p=128
    )
    nc.scalar.wait_ge(dmaP, TOT_STORES)   # tables complete before sweep loads
    nc.vector.wait_ge(dmaW, 48)
    nOS = 0
    for t in range(2 * NSB):
        cl, sb = t // NSB, t % NSB
        tbl = cA if cl == 0 else cB
        own = tbl[0:CELLS].rearrange("(sb j p) e -> sb p j e", sb=NSB, j=J, p=128)
        nxt = tbl[1 : CELLS + 1].rearrange(
            "(sb j p) e -> sb p j e", sb=NSB, j=J, p=128
        )
        s2 = t % 2
        TLv = TL[s2][:].rearrange("p (j h e) -> p j h e", h=2, e=128)
        if t >= 2:
            nc.scalar.wait_ge(evS, t - 1)
        nc.scalar.dma_start(out=TLv[:, :, 0, :], in_=own[sb]).then_inc(dmaS, 16)
        nc.scalar.dma_start(out=TLv[:, :, 1, :], in_=nxt[sb]).then_inc(dmaS, 16)
        nc.vector.wait_ge(dmaS, 32 * (t + 1))
        if t >= 2:
            nc.vector.wait_ge(dmaOS, 16 * (t - 1))
        TLx = TL[s2][:].rearrange(
            "p (j h r x c) -> p j h r x c", h=2, r=2, x=2, c=C
        )
        OSv = ObS[s2][:].rearrange("p (j xr k c) -> p j xr k c", xr=2, k=K, c=C)
        UvS = UbS[:].rearrange("p (j r x c) -> p j r x c", r=2, x=2, c=C)
        Uv4S = UbS[:].rearrange("p (j r c) -> p j r c", r=4, c=C)
        U2vS = U2S[:].rearrange("p (j r c) -> p j r c", r=2, c=C)
        for xr in range(2):
            for k in range(K):
                Wk = wsv[:, cl, sb, :, xr, k, :]
                if xr == 0:
                    Wb = (
                        Wk.rearrange("p j (r x) -> p j r x", r=2, x=2)
                        .unsqueeze(-1)
                        .broadcast_to([128, J, 2, 2, C])
                    )
                    nc.vector.tensor_tensor(
                        out=UvS, in0=TLx[:, :, 0], in1=Wb, op=M
                    )
                    nc.vector.tensor_tensor(
                        out=U2vS, in0=UvS[:, :, :, 0, :], in1=UvS[:, :, :, 1, :],
                        op=A,
                    )
                else:
                    Wk4 = Wk.rearrange("p j (r x) -> p j r x", r=2, x=2)
                    WA = Wk4[:, :, :, 0].unsqueeze(-1).broadcast_to([128, J, 2, C])
                    WB = Wk4[:, :, :, 1].unsqueeze(-1).broadcast_to([128, J, 2, C])
                    nc.vector.tensor_tensor(
                        out=Uv4S[:, :, 0:2, :], in0=TLx[:, :, 0, :, 1, :], in1=WA,
                        op=M,
                    )
                    nc.vector.tensor_tensor(
                        out=Uv4S[:, :, 2:4, :], in0=TLx[:, :, 1, :, 0, :], in1=WB,
                        op=M,
                    )
                    nc.vector.tensor_tensor(
                        out=U2vS, in0=Uv4S[:, :, 0:2, :], in1=Uv4S[:, :, 2:4, :],
                        op=A,
                    )
                fin = nc.vector.tensor_tensor(
                    out=OSv[:, :, xr, k, :], in0=U2vS[:, :, 0, :],
                    in1=U2vS[:, :, 1, :], op=A,
                )
                if xr == 1 and k == K - 1:
                    fin.then_inc(evS, 1)
        nc.sync.wait_ge(evS, t + 1)
        nc.sync.dma_start(out=outSv[cl, sb], in_=ObS[s2][:]).then_inc(dmaOS, 16)
        nOS += 16
    nc.sync.wait_ge(dmaOS, nOS)

    # ---- leftover gather + blend ----
    nc.gpsimd.wait_ge(dmaP, TOT_STORES)
    nc.gpsimd.wait_ge(dmaW, 48)

    wv = w_t[:].rearrange("p (g j) -> p g j", j=4)
    outv = out_d[:].rearrange("(g p) c -> p g c", p=128)
    M = mybir.AluOpType.mult
    A = mybir.AluOpType.add

    nG = 0
    nO = 0
    bglob = 0
    goff = 0
    for si, (tbl, xr) in enumerate([(cA, 0), (cA, 1), (cB, 0), (cB, 1)]):
        GS = gs[si]
        elen = 128 if xr == 0 else 256
        for b in range(GS // BATCH):
            s = bglob % 2
            if bglob >= 2:
                nc.gpsimd.wait_ge(evB, bglob - 1)
            for gi in range(BATCH):
                g = goff + b * BATCH + gi
                nc.gpsimd.indirect_dma_start(
                    out=Gb[s][:, gi * 256 : gi * 256 + elen],
                    out_offset=None,
                    in_=tbl[:],
                    in_offset=IndirectOffsetOnAxis(
                        ap=idx_t[:, g : g + 1], axis=0
                    ),
                ).then_inc(dmaG, 16)
                nG += 16
            nc.vector.wait_ge(dmaG, nG)
            if bglob >= 2:
                nc.vector.wait_ge(dmaO, 16 * (bglob - 1))
            gsl = slice(goff + b * BATCH, goff + b * BATCH + BATCH)
            # Gb viewed [p, g, cell(2), r(2), x(2), c]
            Gv = Gb[s][:].rearrange(
                "p (g cl r x c) -> p g cl r x c", cl=2, r=2, x=2, c=C
            )
            wv4 = w_t[:].rearrange("p (g r x) -> p g r x", r=2, x=2)
            U2v = U2[:].rearrange("p (g r c) -> p g r c", r=2, c=C)
            Ov = Ob[s][:].rearrange("p (g c) -> p g c", c=C)
            if xr == 0:
                Uv = Ub[:].rearrange("p (g r x c) -> p g r x c", r=2, x=2, c=C)
                Wb = (
                    wv4[:, gsl, :, :]
                    .unsqueeze(-1)
                    .broadcast_to([128, BATCH, 2, 2, C])
                )
                nc.vector.tensor_tensor(out=Uv, in0=Gv[:, :, 0], in1=Wb, op=M)
                nc.vector.tensor_tensor(
                    out=U2v, in0=Uv[:, :, :, 0, :], in1=Uv[:, :, :, 1, :], op=A
                )
            else:
                # x0 pixel = (cell0, x=1) with weight w[:, :, 0]; x0+1 = (cell1, x=0)
                GvA = Gv[:, :, 0, :, 1, :]
                GvB = Gv[:, :, 1, :, 0, :]
                WA = (
                    wv4[:, gsl, :, 0].unsqueeze(-1).broadcast_to([128, BATCH, 2, C])
                )
                WB = (
                    wv4[:, gsl, :, 1].unsqueeze(-1).broadcast_to([128, BATCH, 2, C])
                )
                Uv4 = Ub[:].rearrange("p (g r c) -> p g r c", r=4, c=C)
                T1 = Uv4[:, :, 0:2, :]
                T2 = Uv4[:, :, 2:4, :]
                nc.vector.tensor_tensor(out=T1, in0=GvA, in1=WA, op=M)
                nc.vector.tensor_tensor(out=T2, in0=GvB, in1=WB, op=M)
                nc.vector.tensor_tensor(out=U2v, in0=T1, in1=T2, op=A)
            nc.vector.tensor_tensor(
                out=Ov, in0=U2v[:, :, 0, :], in1=U2v[:, :, 1, :], op=A
            ).then_inc(evB, 1)
            nc.sync.wait_ge(evB, bglob + 1)
            nc.sync.dma_start(out=outv[:, gsl, :], in_=Ob[s][:]).then_inc(dmaO, 16)
            nO += 16
            bglob += 1
        goff += GS
    nc.sync.wait_ge(dmaO, nO)
    return nc


_NC_CACHE = {}


def _get_nc(gs):
    if gs not in _NC_CACHE:
        _NC_CACHE[gs] = _build_program(gs)
    return _NC_CACHE[gs]


def _prep_host(pts, g0, g1, g2, g3):
    f = np.float32
    g0s = np.ascontiguousarray(g0[0].transpose(1, 2, 0)) * f(0.125)
    g3s = np.ascontiguousarray(g3[0].transpose(1, 2, 3, 0)) * f(0.6 * 0.125)
    g2s = np.ascontiguousarray(g2[0].transpose(1, 2, 3, 0)) * f(0.4 * 0.25)
    g1s = np.ascontiguousarray(g1[0].transpose(1, 2, 3, 0)) * f(0.2 * 0.5)

    n = pts.shape[0]
    W1 = f(H - 1)
    x = np.clip((pts[:, 0] + f(1.0)) * f(0.5) * W1, f(0.0), W1)
    y = np.clip((pts[:, 1] + f(1.0)) * f(0.5) * W1, f(0.0), W1)
    x0 = np.floor(x)
    y0 = np.floor(y)
    wx = (x - x0).astype(f)
    wy = (y - y0).astype(f)
    x0i = x0.astype(np.int64)
    y0i = y0.astype(np.int64)
    sx = x0i == H - 1
    x0i = np.where(sx, x0i - 1, x0i)
    wx = np.where(sx, f(1.0), wx)
    sy = y0i == H - 1
    y0i = np.where(sy, y0i - 1, y0i)
    wy = np.where(sy, f(1.0), wy)

    core = (y0i >> 7).astype(np.int32)
    yl = (y0i & 127).astype(np.int32)
    P = yl & 1
    k = yl >> 1
    u = (x0i >> 1).astype(np.int32)
    xr = (x0i & 1).astype(np.int32)
    cell = k * 512 + u
    stream = P * 2 + xr
    w4 = np.stack(
        [(1 - wy) * (1 - wx), (1 - wy) * wx, wy * (1 - wx), wy * wx], axis=1
    ).astype(np.float16)

    order = np.lexsort((cell, stream, core))
    cell_s = cell[order]
    stream_s = stream[order]
    core_s = core[order]
    w4_s = w4[order]
    P_s = stream_s >> 1
    xr_s = stream_s & 1

    # rank within (core, P, cell, xr) bin; first K go to sweep slots
    nn = cell_s.shape[0]
    binid = (((core_s.astype(np.int64) * 2 + P_s) * CELLS + cell_s) * 2 + xr_s)
    newb = np.empty(nn, bool)
    newb[0] = True
    newb[1:] = binid[1:] != binid[:-1]
    first = np.maximum.accumulate(np.where(newb, np.arange(nn), 0))
    rank = (np.arange(nn) - first).astype(np.int32)
    slot = rank < K

    # per (core, stream) leftover counts
    counts = np.zeros((NCORES, 4), np.int64)
    for c in range(NCORES):
        mc = (core_s == c) & ~slot
        for s in range(4):
            counts[c, s] = int(np.sum(mc & (stream_s == s)))
    # SPMD: shared group counts per stream = max over cores, batch-rounded
    gs = tuple(
        max(BATCH,
            int(-(-int(counts[:, s].max()) // (128 * BATCH)) * BATCH))
        for s in range(4)
    )
    GTOT = sum(gs)

    # coefficient slabs per core (zero-padded beyond grid)
    def slab(arr, r0, nr, full):
        if arr.ndim == 4:
            out = np.zeros((3, nr) + arr.shape[2:], np.float16)
            hi = min(full, r0 + nr)
            out[:, : hi - r0] = arr[:, r0:hi].astype(np.float16)
        else:
            out = np.zeros((nr,) + arr.shape[1:], np.float16)
            hi = min(full, r0 + nr)
            out[: hi - r0] = arr[r0:hi].astype(np.float16)
        return out

    in_maps = []
    for c in range(NCORES):
        idx2 = np.zeros((128, GTOT), np.int32)
        wt = np.zeros((128, GTOT, 4), np.float16)
        # sweep slot weights [p, cl, sb, j, xr, k, 4]
        wS = np.zeros((128, 2, NSB, J, 2, K, 4), np.float16)
        msl = (core_s == c) & slot
        ce = cell_s[msl]
        wS[ce & 127, P_s[msl], ce // (J * 128), (ce >> 7) % J, xr_s[msl],
           rank[msl]] = w4_s[msl]
        goff = 0
        for s in range(4):
            sel = (core_s == c) & (stream_s == s) & ~slot
            cells_cs = cell_s[sel]
            w_cs = w4_s[sel]
            cnt = cells_cs.shape[0]
            cap = gs[s] * 128
            assert cnt <= cap, f"stream overflow core {c} stream {s}"
            # point j -> group goff + j//128, partition j%128
            gidx = goff + np.arange(cnt) // 128
            pidx = np.arange(cnt) % 128
            idx2[pidx, gidx] = cells_cs
            wt[pidx, gidx] = w_cs
            goff += gs[s]
        in_maps.append(
            {
                "gA": np.concatenate(
                    [slab(g0s, 16 * c, 18, 128)[None], slab(g3s, 16 * c, 18, 128)]
                ),
                "ll2b": np.concatenate(
                    [np.zeros((1, 36, 256, C), np.float16),
                     slab(g2s, 32 * c, 36, 256)]
                ),
                "ll1b": np.concatenate(
                    [np.zeros((1, 72, 512, C), np.float16),
                     slab(g1s, 64 * c, 72, 512)]
                ),
                "idx": idx2,
                "w": np.ascontiguousarray(wt.reshape(128, 4 * GTOT)),
                "ws": np.ascontiguousarray(wS.reshape(128, -1)),
            }
        )
    return in_maps, order, counts, gs, n, (core_s, P_s, xr_s, cell_s, rank, slot)


def kernel(pts, g0, g1, g2, g3, _res_hook=None):
    pts = np.asarray(pts, np.float32)
    in_maps, order, counts, gs, n, meta = _prep_host(
        pts, np.asarray(g0, np.float32), np.asarray(g1, np.float32),
        np.asarray(g2, np.float32), np.asarray(g3, np.float32),
    )
    core_s, P_s, xr_s, cell_s, rank, slot = meta
    nc = _get_nc(gs)
    res = bass_utils.run_bass_kernel_spmd(nc, in_maps, list(range(NCORES)))
    if _res_hook is not None:
        _res_hook(res)
    out_sorted = np.empty((n, C), np.float32)
    for c in range(NCORES):
        mc = core_s == c
        # sweep-slotted points
        oS = res.results[c]["outS"].reshape(2, CELLS, 2, K, C)
        msl = mc & slot
        out_sorted[msl] = oS[
            P_s[msl], cell_s[msl], xr_s[msl], rank[msl]
        ].astype(np.float32)
        # leftover points, packed per stream in sorted order
        o = res.results[c]["out"]
        goff = 0
        for s in range(4):
            sel = mc & ~slot & ((P_s * 2 + xr_s) == s)
            cnt = int(counts[c, s])
            out_sorted[sel] = o[goff * 128 : goff * 128 + cnt].astype(np.float32)
            goff += gs[s]
    full = np.empty_like(out_sorted)
    full[order] = out_sorted
    return full


# revision 23
# speedup vs baseline: 6.3510x; 1.0016x over previous
import sys

sys.path.insert(0, "/opt/trn_rl_repo")
import numpy as np
from concourse import mybir
from concourse.bass import Bass, IndirectOffsetOnAxis
from concourse import bass_utils

C = 32
H = 1024
NCORES = 8
BATCH = 16            # groups per blend batch
CELLS = 64 * 512      # pair-cells per class table per core
SLAB = 144            # local plane rows computed per core (18 coarse rows)
K = 3                 # capacity slots per (cell, xr) in the sweep
J = 32                # cell-blocks (of 128 cells) per sweep iteration
NSB = CELLS // (J * 128)   # sweep iterations per class table

_f16 = mybir.dt.float16
_i32 = mybir.dt.int32


def _build_program(gs):
    """gs: tuple of 4 group counts (classA/xr0, classA/xr1, classB/xr0, classB/xr1),
    each a multiple of BATCH. One SPMD program; per-core data via in_maps."""
    GTOT = sum(gs)
    nc = Bass()
    gA = nc.declare_dram_parameter("gA", [4, 18, 128, C], _f16, isOutput=False)
    ll2b = nc.declare_dram_parameter("ll2b", [4, 36, 256, C], _f16, isOutput=False)
    ll1b = nc.declare_dram_parameter("ll1b", [4, 72, 512, C], _f16, isOutput=False)
    idx_d = nc.declare_dram_parameter("idx", [128, GTOT], _i32, isOutput=False)
    w_d = nc.declare_dram_parameter("w", [128, 4 * GTOT], _f16, isOutput=False)
    # sweep slot weights: [p, cl, sb, j, xr, k, 4]
    WTOT = 2 * NSB * J * 2 * K * 4
    ws_d = nc.declare_dram_parameter("ws", [128, WTOT], _f16, isOutput=False)
    out_d = nc.declare_dram_parameter("out", [GTOT * 128, C], _f16, isOutput=True)
    outS_d = nc.declare_dram_parameter(
        "outS", [2 * CELLS, 2 * K * C], _f16, isOutput=True
    )

    cA = nc.dram_tensor("cA", [CELLS + 1, 2 * 2 * C], _f16)  # [k*512+u, (r,x,c)]
    cB = nc.dram_tensor("cB", [CELLS + 1, 2 * 2 * C], _f16)

    dmaL = nc.alloc_semaphore()   # idwt input loads
    dmaP = nc.alloc_semaphore()   # idwt stores
    evW = nc.alloc_semaphore()    # idwt compute iters
    dmaW = nc.alloc_semaphore()   # idx/w loads
    dmaG = nc.alloc_semaphore()   # indirect gathers (must start at 0)
    evB = nc.alloc_semaphore()    # blend batches
    dmaO = nc.alloc_semaphore()   # output stores
    dmaS = nc.alloc_semaphore()   # sweep table loads
    evS = nc.alloc_semaphore()    # sweep blend iterations
    dmaOS = nc.alloc_semaphore()  # sweep output stores

    # ---- SBUF ----
    CHW = 64 * C                  # 64 output-col-pairs worth of one parity = 2048 els
    inb = [nc.alloc_sbuf_tensor(f"in{s}", [128, 4 * CHW], _f16).ap()
           for s in range(2)]
    tmp = [[nc.alloc_sbuf_tensor(f"tmp{s}_{k}", [128, CHW], _f16).ap()
            for k in range(4)] for s in range(2)]
    # out bigbuf layout [p, w(64), rp(2), xp(2), c]
    outb = [nc.alloc_sbuf_tensor(f"out{s}", [128, 4 * CHW], _f16).ap()
            for s in range(2)]
    idx_t = nc.alloc_sbuf_tensor("idx_t", [128, GTOT], _i32).ap()
    w_t = nc.alloc_sbuf_tensor("w_t", [128, 4 * GTOT], _f16).ap()
    Gb = [nc.alloc_sbuf_tensor(f"G{s}", [128, BATCH * 256], _f16).ap() for s in range(2)]
    Ub = nc.alloc_sbuf_tensor("Ub", [128, BATCH * 4 * C], _f16).ap()
    U2 = nc.alloc_sbuf_tensor("U2", [128, BATCH * 2 * C], _f16).ap()
    Ob = [nc.alloc_sbuf_tensor(f"O{s}", [128, BATCH * C], _f16).ap() for s in range(2)]
    ws_t = nc.alloc_sbuf_tensor("ws_t", [128, WTOT], _f16).ap()
    TL = [inb[0], outb[0]]  # reuse idwt buffers (idle once tables are stored)
    UbS = nc.alloc_sbuf_tensor("UbS", [128, J * 4 * C], _f16).ap()
    U2S = nc.alloc_sbuf_tensor("U2S", [128, J * 2 * C], _f16).ap()
    ObS = [nc.alloc_sbuf_tensor(f"OS{s}", [128, J * 2 * K * C], _f16).ap()
           for s in range(2)]

    # ---- IDWT ----
    # levels: A: 18 coarse rows,128 wide -> ll2 (36,256); 2 col chunks of 64
    #         B: 36 rows,256 wide -> ll1 (72,512); 4 chunks
    #         C: 72 rows,512 wide -> cA/cB tables; 8 chunks
    iters = [("A", cb) for cb in range(2)] + [("B", cb) for cb in range(4)] + \
            [("C", cb) for cb in range(8)]
    NIT = len(iters)
    NPL = {"A": 18, "B": 36, "C": 72}
    ll2v = ll2b[0].rearrange("(p two) (w xp) c -> p two w xp c", two=2, xp=2)
    ll1v = ll1b[0].rearrange("(p two) (w xp) c -> p two w xp c", two=2, xp=2)
    cAv = cA[0:CELLS].rearrange("(k u) (r x c) -> k u r x c", u=512, r=2, x=2)
    cBv = cB[0:CELLS].rearrange("(k u) (r x c) -> k u r x c", u=512, r=2, x=2)

    def src_packed(level, cb):
        cs = slice(cb * 64, cb * 64 + 64)
        if level == "A":
            return gA[:, :, cs, :].rearrange("b p w c -> p b w c")
        if level == "B":
            return ll2b[:, :, cs, :].rearrange("b p w c -> p b w c")
        return ll1b[:, :, cs, :].rearrange("b p w c -> p b w c")

    stores_per_iter = {"A": 4, "B": 4, "C": 3}
    cum_stores = []
    tot = 0
    for lv, _ in iters:
        cum_stores.append(tot)
        tot += stores_per_iter[lv] * 16
    TOT_STORES = tot
    lvlB_start, lvlC_start = 2, 6

    nL = 0
    nW = 0
    for i, (level, cb) in enumerate(iters):
        s = i % 2
        NP = NPL[level]
        if i >= 2:
            nc.sync.wait_ge(evW, i - 1)
        if i == lvlB_start:
            nc.sync.wait_ge(dmaP, cum_stores[lvlB_start])
        if i == lvlC_start:
            nc.sync.wait_ge(dmaP, cum_stores[lvlC_start])
        nc.sync.dma_start(
            out=inb[s][:NP].rearrange("p (b w c) -> p b w c", b=4, c=C),
            in_=src_packed(level, cb),
        ).then_inc(dmaL, 16)
        nL += 16
        nc.vector.wait_ge(dmaL, nL)
        if i >= 2:
            nc.vector.wait_ge(dmaP, cum_stores[i - 1])  # stores of iter i-2 done
        A = mybir.AluOpType.add
        S = mybir.AluOpType.subtract
        inv = inb[s][:NP].rearrange("p (b e) -> p b e", b=4)
        ll, lh, hl, hh = (inv[:, k] for k in range(4))
        t1, t2, t3, t4 = (b[:NP] for b in tmp[s])
        ov = outb[s][:NP].rearrange("p (w rp xp c) -> p w rp xp c", rp=2, xp=2, c=C)
        oEE = ov[:, :, 0, 0, :]
        oEO = ov[:, :, 0, 1, :]
        oOE = ov[:, :, 1, 0, :]
        oOO = ov[:, :, 1, 1, :]
        nc.vector.tensor_tensor(out=t1, in0=ll, in1=lh, op=S)   # row-even lo
        nc.vector.tensor_tensor(out=t2, in0=ll, in1=lh, op=A)   # row-odd lo
        nc.vector.tensor_tensor(out=t3, in0=hl, in1=hh, op=S)   # row-even hi
        nc.vector.tensor_tensor(out=t4, in0=hl, in1=hh, op=A)   # row-odd hi
        nc.vector.tensor_tensor(out=oEE, in0=t1, in1=t3, op=S)  # (2r, 2w)
        nc.vector.tensor_tensor(out=oEO, in0=t1, in1=t3, op=A)  # (2r, 2w+1)
        nc.vector.tensor_tensor(out=oOE, in0=t2, in1=t4, op=S)  # (2r+1, 2w)
        nc.vector.tensor_tensor(out=oOO, in0=t2, in1=t4, op=A).then_inc(evW, 1)
        nW += 1
        nc.scalar.wait_ge(evW, nW)
        ws = slice(cb * 64, cb * 64 + 64)
        bufv = outb[s][:].rearrange("p (w rp xp c) -> p w rp xp c", rp=2, xp=2, c=C)
        if level in ("A", "B"):
            dstv = ll2v if level == "A" else ll1v
            for rp in range(2):
                for xp in range(2):
                    nc.scalar.dma_start(
                        out=dstv[:NP, rp, ws, xp, :],
                        in_=bufv[:NP, :, rp, xp, :],
                    ).then_inc(dmaP, 16)
        else:
            # classA: pair k=p rows (2p, 2p+1): full cells, contiguous
            nc.scalar.dma_start(
                out=cAv[0:64, ws, :, :, :], in_=bufv[0:64]
            ).then_inc(dmaP, 16)
            # classB r0 = odd rows (2p+1): k=p
            nc.scalar.dma_start(
                out=cBv[0:64, ws, 0, :, :], in_=bufv[0:64, :, 1, :, :]
            ).then_inc(dmaP, 16)
            # classB r1 = even rows (2p), p=1..64 -> k=p-1
            nc.scalar.dma_start(
                out=cBv[0:64, ws, 1, :, :], in_=bufv[1:65, :, 0, :, :]
            ).then_inc(dmaP, 16)

    # ---- sweep: K slots per (cell, xr), plain DMA loads, static-AP blends ----
    nc.sync.dma_start(out=idx_t[:], in_=idx_d[:]).then_inc(dmaW, 16)
    nc.sync.dma_start(out=w_t[:], in_=w_d[:]).then_inc(dmaW, 16)
    nc.sync.dma_start(out=ws_t[:], in_=ws_d[:]).then_inc(dmaW, 16)
    M = mybir.AluOpType.mult
    A = mybir.AluOpType.add
    wsv = ws_t[:].rearrange(
        "p (cl sb j xr k q) -> p cl sb j xr k q", cl=2, sb=NSB, j=J, xr=2, k=K
    )
    outSv = outS_d[:].rearrange(
        "(cl sb j p) w -> cl sb p j w", cl=2, sb=NSB, j=J, p=128
    )
    nc.scalar.wait_ge(dmaP, TOT_STORES)   # tables complete before sweep loads
    nc.vector.wait_ge(dmaW, 48)
    # gpsimd: issue all leftover gathers up front (paced by evB vs blends)
    nc.gpsimd.wait_ge(dmaP, TOT_STORES)
    nc.gpsimd.wait_ge(dmaW, 48)
    wv4 = w_t[:].rearrange("p (g r x) -> p g r x", r=2, x=2)
    outv = out_d[:].rearrange("(g p) c -> p g c", p=128)
    batches = []
    nG = 0
    bglob = 0
    goff = 0
    for si, (tblx, xr) in enumerate([(cA, 0), (cA, 1), (cB, 0), (cB, 1)]):
        GS = gs[si]
        elen = 128 if xr == 0 else 256
        for b in range(GS // BATCH):
            s = bglob % 2
            if bglob >= 2:
                nc.gpsimd.wait_ge(evB, bglob - 1)
            for gi in range(BATCH):
                g = goff + b * BATCH + gi
                nc.gpsimd.indirect_dma_start(
                    out=Gb[s][:, gi * 256 : gi * 256 + elen],
                    out_offset=None,
                    in_=tblx[:],
                    in_offset=IndirectOffsetOnAxis(
                        ap=idx_t[:, g : g + 1], axis=0
                    ),
                ).then_inc(dmaG, 16)
                nG += 16
            batches.append((bglob, s, goff + b * BATCH, xr, nG))
            bglob += 1
        goff += GS
    NBATCH = bglob
    nO = [0]

    def emit_blend(entry):
        bg, s, gb0, xr, nGb = entry
        nc.vector.wait_ge(dmaG, nGb)
        if bg >= 2:
            nc.vector.wait_ge(dmaO, 16 * (bg - 1))
        gsl = slice(gb0, gb0 + BATCH)
        Gv = Gb[s][:].rearrange(
            "p (g cl r x c) -> p g cl r x c", cl=2, r=2, x=2, c=C
        )
        U2v = U2[:].rearrange("p (g r c) -> p g r c", r=2, c=C)
        Ov = Ob[s][:].rearrange("p (g c) -> p g c", c=C)
        if xr == 0:
            Uv = Ub[:].rearrange("p (g r x c) -> p g r x c", r=2, x=2, c=C)
            Wb = (
                wv4[:, gsl, :, :]
                .unsqueeze(-1)
                .broadcast_to([128, BATCH, 2, 2, C])
            )
            nc.vector.tensor_tensor(out=Uv, in0=Gv[:, :, 0], in1=Wb, op=M)
            nc.vector.tensor_tensor(
                out=U2v, in0=Uv[:, :, :, 0, :], in1=Uv[:, :, :, 1, :], op=A
            )
        else:
            GvA = Gv[:, :, 0, :, 1, :]
            GvB = Gv[:, :, 1, :, 0, :]
            WA = wv4[:, gsl, :, 0].unsqueeze(-1).broadcast_to([128, BATCH, 2, C])
            WB = wv4[:, gsl, :, 1].unsqueeze(-1).broadcast_to([128, BATCH, 2, C])
            Uv4 = Ub[:].rearrange("p (g r c) -> p g r c", r=4, c=C)
            T1 = Uv4[:, :, 0:2, :]
            T2 = Uv4[:, :, 2:4, :]
            nc.vector.tensor_tensor(out=T1, in0=GvA, in1=WA, op=M)
            nc.vector.tensor_tensor(out=T2, in0=GvB, in1=WB, op=M)
            nc.vector.tensor_tensor(out=U2v, in0=T1, in1=T2, op=A)
        nc.vector.tensor_tensor(
            out=Ov, in0=U2v[:, :, 0, :], in1=U2v[:, :, 1, :], op=A
        ).then_inc(evB, 1)
        nc.sync.wait_ge(evB, bg + 1)
        nc.sync.dma_start(out=outv[:, gsl, :], in_=Ob[s][:]).then_inc(dmaO, 16)
        nO[0] += 16

    # ---- sweep with interleaved leftover blends ----
    nOS = 0
    emitted = 0
    NSW = 2 * NSB
    for t in range(2 * NSB):
        cl, sb = t // NSB, t % NSB
        tbl = cA if cl == 0 else cB
        own = tbl[0:CELLS].rearrange("(sb j p) e -> sb p j e", sb=NSB, j=J, p=128)
        nxt = tbl[1 : CELLS + 1].rearrange(
            "(sb j p) e -> sb p j e", sb=NSB, j=J, p=128
        )
        s2 = t % 2
        TLv = TL[s2][:].rearrange("p (j h e) -> p j h e", h=2, e=128)
        if t >= 2:
            nc.scalar.wait_ge(evS, t - 1)
        nc.scalar.dma_start(out=TLv[:, :, 0, :], in_=own[sb]).then_inc(dmaS, 16)
        nc.scalar.dma_start(out=TLv[:, :, 1, :], in_=nxt[sb]).then_inc(dmaS, 16)
        nc.vector.wait_ge(dmaS, 32 * (t + 1))
        if t >= 2:
            nc.vector.wait_ge(dmaOS, 16 * (t - 1))
        TLx = TL[s2][:].rearrange(
            "p (j h r x c) -> p j h r x c", h=2, r=2, x=2, c=C
        )
        OSv = ObS[s2][:].rearrange("p (j xr k c) -> p j xr k c", xr=2, k=K, c=C)
        UvS = UbS[:].rearrange("p (j r x c) -> p j r x c", r=2, x=2, c=C)
        Uv4S = UbS[:].rearrange("p (j r c) -> p j r c", r=4, c=C)
        U2vS = U2S[:].rearrange("p (j r c) -> p j r c", r=2, c=C)
        for xr in range(2):
            for k in range(K):
                Wk = wsv[:, cl, sb, :, xr, k, :]
                if xr == 0:
                    Wb = (
                        Wk.rearrange("p j (r x) -> p j r x", r=2, x=2)
                        .unsqueeze(-1)
                        .broadcast_to([128, J, 2, 2, C])
                    )
                    nc.vector.tensor_tensor(
                        out=UvS, in0=TLx[:, :, 0], in1=Wb, op=M
                    )
                    nc.vector.tensor_tensor(
                        out=U2vS, in0=UvS[:, :, :, 0, :], in1=UvS[:, :, :, 1, :],
                        op=A,
                    )
                else:
                    Wk4 = Wk.rearrange("p j (r x) -> p j r x", r=2, x=2)
                    WA = Wk4[:, :, :, 0].unsqueeze(-1).broadcast_to([128, J, 2, C])
                    WB = Wk4[:, :, :, 1].unsqueeze(-1).broadcast_to([128, J, 2, C])
                    nc.vector.tensor_tensor(
                        out=Uv4S[:, :, 0:2, :], in0=TLx[:, :, 0, :, 1, :], in1=WA,
                        op=M,
                    )
                    nc.vector.tensor_tensor(
                        out=Uv4S[:, :, 2:4, :], in0=TLx[:, :, 1, :, 0, :], in1=WB,
                        op=M,
                    )
                    nc.vector.tensor_tensor(
                        out=U2vS, in0=Uv4S[:, :, 0:2, :], in1=Uv4S[:, :, 2:4, :],
                        op=A,
                    )
                fin = nc.vector.tensor_tensor(
                    out=OSv[:, :, xr, k, :], in0=U2vS[:, :, 0, :],
                    in1=U2vS[:, :, 1, :], op=A,
                )
                if xr == 1 and k == K - 1:
                    fin.then_inc(evS, 1)
        nc.sync.wait_ge(evS, t + 1)
        nc.sync.dma_start(out=outSv[cl, sb], in_=ObS[s2][:]).then_inc(dmaOS, 16)
        nOS += 16
        while emitted < 0:
            emit_blend(batches[emitted])
            emitted += 1
    while emitted < NBATCH:
        emit_blend(batches[emitted])
        emitted += 1
    nc.sync.wait_ge(dmaOS, nOS)
    nc.sync.wait_ge(dmaO, nO[0])
    return nc


_NC_CACHE = {}


def _get_nc(gs):
    if gs not in _NC_CACHE:
        _NC_CACHE[gs] = _build_program(gs)
    return _NC_CACHE[gs]


def _prep_host(pts, g0, g1, g2, g3):
    f = np.float32
    g0s = np.ascontiguousarray(g0[0].transpose(1, 2, 0)) * f(0.125)
    g3s = np.ascontiguousarray(g3[0].transpose(1, 2, 3, 0)) * f(0.6 * 0.125)
    g2s = np.ascontiguousarray(g2[0].transpose(1, 2, 3, 0)) * f(0.4 * 0.25)
    g1s = np.ascontiguousarray(g1[0].transpose(1, 2, 3, 0)) * f(0.2 * 0.5)

    n = pts.shape[0]
    W1 = f(H - 1)
    x = np.clip((pts[:, 0] + f(1.0)) * f(0.5) * W1, f(0.0), W1)
    y = np.clip((pts[:, 1] + f(1.0)) * f(0.5) * W1, f(0.0), W1)
    x0 = np.floor(x)
    y0 = np.floor(y)
    wx = (x - x0).astype(f)
    wy = (y - y0).astype(f)
    x0i = x0.astype(np.int64)
    y0i = y0.astype(np.int64)
    sx = x0i == H - 1
    x0i = np.where(sx, x0i - 1, x0i)
    wx = np.where(sx, f(1.0), wx)
    sy = y0i == H - 1
    y0i = np.where(sy, y0i - 1, y0i)
    wy = np.where(sy, f(1.0), wy)

    core = (y0i >> 7).astype(np.int32)
    yl = (y0i & 127).astype(np.int32)
    P = yl & 1
    k = yl >> 1
    u = (x0i >> 1).astype(np.int32)
    xr = (x0i & 1).astype(np.int32)
    cell = k * 512 + u
    stream = P * 2 + xr
    w4 = np.stack(
        [(1 - wy) * (1 - wx), (1 - wy) * wx, wy * (1 - wx), wy * wx], axis=1
    ).astype(np.float16)

    order = np.lexsort((cell, stream, core))
    cell_s = cell[order]
    stream_s = stream[order]
    core_s = core[order]
    w4_s = w4[order]
    P_s = stream_s >> 1
    xr_s = stream_s & 1

    # rank within (core, P, cell, xr) bin; first K go to sweep slots
    nn = cell_s.shape[0]
    binid = (((core_s.astype(np.int64) * 2 + P_s) * CELLS + cell_s) * 2 + xr_s)
    newb = np.empty(nn, bool)
    newb[0] = True
    newb[1:] = binid[1:] != binid[:-1]
    first = np.maximum.accumulate(np.where(newb, np.arange(nn), 0))
    rank = (np.arange(nn) - first).astype(np.int32)
    slot = rank < K

    # per (core, stream) leftover counts
    counts = np.zeros((NCORES, 4), np.int64)
    for c in range(NCORES):
        mc = (core_s == c) & ~slot
        for s in range(4):
            counts[c, s] = int(np.sum(mc & (stream_s == s)))
    # SPMD: shared group counts per stream = max over cores, batch-rounded
    gs = tuple(
        max(BATCH,
            int(-(-int(counts[:, s].max()) // (128 * BATCH)) * BATCH))
        for s in range(4)
    )
    GTOT = sum(gs)

    # coefficient slabs per core (zero-padded beyond grid)
    def slab(arr, r0, nr, full):
        if arr.ndim == 4:
            out = np.zeros((3, nr) + arr.shape[2:], np.float16)
            hi = min(full, r0 + nr)
            out[:, : hi - r0] = arr[:, r0:hi].astype(np.float16)
        else:
            out = np.zeros((nr,) + arr.shape[1:], np.float16)
            hi = min(full, r0 + nr)
            out[: hi - r0] = arr[r0:hi].astype(np.float16)
        return out

    in_maps = []
    for c in range(NCORES):
        idx2 = np.zeros((128, GTOT), np.int32)
        wt = np.zeros((128, GTOT, 4), np.float16)
        # sweep slot weights [p, cl, sb, j, xr, k, 4]
        wS = np.zeros((128, 2, NSB, J, 2, K, 4), np.float16)
        msl = (core_s == c) & slot
        ce = cell_s[msl]
        wS[ce & 127, P_s[msl], ce // (J * 128), (ce >> 7) % J, xr_s[msl],
           rank[msl]] = w4_s[msl]
        goff = 0
        for s in range(4):
            sel = (core_s == c) & (stream_s == s) & ~slot
            cells_cs = cell_s[sel]
            w_cs = w4_s[sel]
            cnt = cells_cs.shape[0]
            cap = gs[s] * 128
            assert cnt <= cap, f"stream overflow core {c} stream {s}"
            # point j -> group goff + j//128, partition j%128
            gidx = goff + np.arange(cnt) // 128
            pidx = np.arange(cnt) % 128
            idx2[pidx, gidx] = cells_cs
            wt[pidx, gidx] = w_cs
            goff += gs[s]
        in_maps.append(
            {
                "gA": np.concatenate(
                    [slab(g0s, 16 * c, 18, 128)[None], slab(g3s, 16 * c, 18, 128)]
                ),
                "ll2b": np.concatenate(
                    [np.zeros((1, 36, 256, C), np.float16),
                     slab(g2s, 32 * c, 36, 256)]
                ),
                "ll1b": np.concatenate(
                    [np.zeros((1, 72, 512, C), np.float16),
                     slab(g1s, 64 * c, 72, 512)]
                ),
                "idx": idx2,
                "w": np.ascontiguousarray(wt.reshape(128, 4 * GTOT)),
                "ws": np.ascontiguousarray(wS.reshape(128, -1)),
            }
        )
    return in_maps, order, counts, gs, n, (core_s, P_s, xr_s, cell_s, rank, slot)


def kernel(pts, g0, g1, g2, g3, _res_hook=None):
    pts = np.asarray(pts, np.float32)
    in_maps, order, counts, gs, n, meta = _prep_host(
        pts, np.asarray(g0, np.float32), np.asarray(g1, np.float32),
        np.asarray(g2, np.float32), np.asarray(g3, np.float32),
    )
    core_s, P_s, xr_s, cell_s, rank, slot = meta
    nc = _get_nc(gs)
    res = bass_utils.run_bass_kernel_spmd(nc, in_maps, list(range(NCORES)))
    if _res_hook is not None:
        _res_hook(res)
    out_sorted = np.empty((n, C), np.float32)
    for c in range(NCORES):
        mc = core_s == c
        # sweep-slotted points
        oS = res.results[c]["outS"].reshape(2, CELLS, 2, K, C)
        msl = mc & slot
        out_sorted[msl] = oS[
            P_s[msl], cell_s[msl], xr_s[msl], rank[msl]
        ].astype(np.float32)
        # leftover points, packed per stream in sorted order
        o = res.results[c]["out"]
        goff = 0
        for s in range(4):
            sel = mc & ~slot & ((P_s * 2 + xr_s) == s)
            cnt = int(counts[c, s])
            out_sorted[sel] = o[goff * 128 : goff * 128 + cnt].astype(np.float32)
            goff += gs[s]
    full = np.empty_like(out_sorted)
    full[order] = out_sorted
    return full
